# revision 44
# baseline (speedup 1.0000x reference)
"""Trainium2 Bass kernel for EnhancedSAGEModel (3x SAGEConv mean-aggr + BN + FFN head).

Sharding: nodes across 8 cores (12500 real + 300 pad slots each); edges
partitioned by destination; per-layer gather of source rows from a replicated
node-feature table (fp16), scatter-add via one-hot matmul into PSUM in
transposed (feature-major) layout; AllGather of activations between layers;
AllReduce for BatchNorm batch statistics.

Self-contained: hardcodes the problem shapes from spec
(x:[100000,128] f32, edge_index:[2,1600000] i64, weights 128x128 etc.).
"""

import math
import os
import numpy as np
import ml_dtypes

import concourse.bass as bass
import concourse.bacc as bacc
import concourse.mybir as mybir
import concourse.tile as tile
from concourse.bass import ts
from concourse import bass_utils

F16 = mybir.dt.float16
F32 = mybir.dt.float32
F8 = mybir.dt.float8e4
NP_F8 = ml_dtypes.float8_e4m3
I16 = mybir.dt.int16

# mm8: one-hot in fp8 (exact), xe fp16, plain matmuls (default)
# dr8: one-hot + xe in fp8, DoubleRow paired matmuls (fast, ~1.4e-2 rel err)
# fp16: everything fp16
MODE = os.environ.get("BASS_GNN_MODE", "mm8")
assert MODE in ("mm8", "dr8", "fp16")
USE_FP8 = MODE == "dr8"          # xe cast + DoubleRow
MM_FP8 = MODE in ("mm8", "dr8")  # one-hot dtype

AX = mybir.AxisListType
ALU = mybir.AluOpType
ACTF = mybir.ActivationFunctionType

EPS = 1e-5


class Cfg:
    """Quarter-major table layout: global table row of node (core k, local i)
    with i in quarter q (by window) is cstart[q] + k*qrows[q] + (i - qsrow[q]).
    Chunk c == quarter c, so the next-layer gather of chunk c only depends on
    the AllGather of quarter c (enables chunked AG overlap with pass_b)."""

    def __init__(self, n_real, e, n_cores=8, d=128, c_out=64):
        self.N = n_real
        self.E = e
        self.NC = n_cores
        self.D = d
        self.C = c_out
        assert n_real % n_cores == 0
        self.PCR = n_real // n_cores              # real nodes per core
        self.NW = math.ceil(self.PCR / 128)       # windows per core (128 dst each)
        self.NPC = self.NW * 128                  # padded nodes per core
        self.NTOT = self.NPC * n_cores            # padded global nodes
        self.NCHUNK = 4
        # front-loaded quarters: the last AllGather (the only one whose wire
        # time is exposed at the layer boundary) covers just a few windows
        qw_max = 32767 // (128 * n_cores)
        last = max(1, self.NW - 3 * qw_max)
        assert last <= qw_max
        self.QW = [qw_max, qw_max, qw_max, last]
        self.QWS = np.concatenate([[0], np.cumsum(self.QW)])  # window starts
        self.qrows = [w * 128 for w in self.QW]               # rows per quarter
        self.qsrow = [int(s) * 128 for s in self.QWS[:-1]]    # local row starts
        self.csize = [n_cores * r for r in self.qrows]        # chunk sizes
        self.cstart = [0]
        for s in self.csize[:-1]:
            self.cstart.append(self.cstart[-1] + s)
        for cs in self.csize:
            assert cs <= 32767, "int16 gather index limit"
        self.NB = math.ceil(self.NW / 4)          # dense blocks (<=4 windows each)
        # pass_b block after which quarter q's h_loc rows are all written
        self.q_done_block = [math.ceil(int(self.QWS[q + 1]) / 4) - 1
                             for q in range(self.NCHUNK)]

    def block_windows(self, b):
        return range(4 * b, min(4 * b + 4, self.NW))


def plan_edges(cfg, edge_index):
    """Bin edges per core by (dense-block, chunk, window); pad each bin to a
    multiple of 128 slots. Returns per-core plan arrays + shared structure."""
    src = np.asarray(edge_index[0], dtype=np.int64)
    dst = np.asarray(edge_index[1], dtype=np.int64)
    NC, PCR, NPC = cfg.NC, cfg.PCR, cfg.NPC
    NW, NCH = cfg.NW, cfg.NCHUNK

    deg = np.bincount(dst, minlength=cfg.N).astype(np.float32)
    invdeg_full = (1.0 / np.maximum(deg, 1.0)).astype(np.float32)

    core = dst // PCR
    ld_all = dst % PCR                       # local dst in [0, PCR)
    w_all = ld_all // 128
    ldw_all = (ld_all % 128).astype(np.float16)
    # quarter-major chunk mapping of sources
    k_src = src // PCR
    i_src = src % PCR
    c_all = np.searchsorted(cfg.QWS[1:], i_src // 128, side="right")
    qrows_a = np.asarray(cfg.qrows, dtype=np.int64)
    qsrow_a = np.asarray(cfg.qsrow, dtype=np.int64)
    lsrc_all = (k_src * qrows_a[c_all] + (i_src - qsrow_a[c_all])).astype(
        np.int16)

    # bin linear order: for block: for chunk: for window-in-block
    # bin_rank(w, c) -> position in that order
    bin_rank = np.empty((NW, NCH), dtype=np.int64)
    r = 0
    order = []
    for b in range(cfg.NB):
        for c in range(NCH):
            for w in cfg.block_windows(b):
                bin_rank[w, c] = r
                order.append((w, c))
                r += 1
    NBINS = r

    counts = np.zeros((NC, NBINS), dtype=np.int64)
    binid = bin_rank[w_all, c_all]
    for k in range(NC):
        m = core == k
        counts[k] = np.bincount(binid[m], minlength=NBINS)

    G = np.maximum(np.ceil(counts.max(axis=0) / 128.0).astype(np.int64), 0)
    g_off = np.concatenate([[0], np.cumsum(G)])  # group offset per bin
    NG = int(g_off[-1])

    mm_np = NP_F8 if MM_FP8 else np.float16
    idx_plans, mm_plans, inv_plans, nreal_plans = [], [], [], []
    for k in range(NC):
        m = core == k
        bk = binid[m]
        lsrc_k = lsrc_all[m]
        ldw_k = ldw_all[m]
        ordk = np.argsort(bk, kind="stable")
        bk = bk[ordk]
        lsrc_k = lsrc_k[ordk]
        ldw_k = ldw_k[ordk].astype(np.int64)
        starts = np.concatenate([[0], np.cumsum(counts[k])])
        rank = np.arange(len(bk)) - starts[bk]

        idx_arr = np.zeros((128, NG * 8), dtype=np.int16)
        # Trailing pad of the LAST nonempty bin of each (block, chunk) gather
        # call: mark idx=-1 and record the per-core real count (passed as
        # num_idxs_reg) so the ucode skips those descriptors at runtime.
        nreal = np.zeros(cfg.NB * NCH, dtype=np.int32)
        for b in range(cfg.NB):
            wins_b = list(cfg.block_windows(b))
            for c in range(NCH):
                gb0 = g_off[bin_rank[wins_b[0], c]]
                ng_call = int(sum(G[bin_rank[w, c]] for w in wins_b))
                if ng_call == 0:
                    continue
                wl = max(w for w in wins_b if G[bin_rank[w, c]] > 0)
                bin_ = bin_rank[wl, c]
                gb, ge = g_off[bin_], g_off[bin_ + 1]
                pad_r = np.arange(counts[k][bin_], (ge - gb) * 128)
                idx_arr[pad_r % 16, gb * 8 + pad_r // 16] = -1
                nreal[b * NCH + c] = (gb - gb0) * 128 + counts[k][bin_]
        col = g_off[bk] * 8 + rank // 16
        idx_arr[rank % 16, col] = lsrc_k
        # HW ucode: each of the 8 Q7 cores reads its own 16-partition group
        idx_arr = np.tile(idx_arr[:16], (8, 1))
        # host-precomputed one-hot scatter matrix: slot (g*128+p) -> dst d
        slot = g_off[bk] * 128 + rank
        mm_arr = np.zeros((128, NG * 128), dtype=mm_np)
        mm_arr[slot % 128, (slot // 128) * 128 + ldw_k] = 1.0

        inv = np.ones((NPC,), dtype=np.float32)
        inv[:PCR] = invdeg_full[k * PCR:(k + 1) * PCR]
        inv_plans.append(inv.reshape(NW, 128))
        idx_plans.append(idx_arr)
        mm_plans.append(mm_arr)
        nreal_plans.append(nreal.reshape(1, -1))

    return dict(G=G, g_off=g_off, NG=NG, bin_rank=bin_rank,
                idx=idx_plans, mmoh=mm_plans, inv=inv_plans, nreal=nreal_plans)


def build_kernel(cfg, G, g_off, NG, bin_rank):
    """Build the 8-core SPMD Bass program. Structure (G table) is compile-time."""
    NW, NCH, NPC, NTOT, NB = cfg.NW, cfg.NCHUNK, cfg.NPC, cfg.NTOT, cfg.NB
    N_real = cfg.N

    nc = bacc.Bacc("TRN2", target_bir_lowering=False, debug=False,
                   num_devices=cfg.NC, num_swdge_queues=4)
    rg = [list(range(cfg.NC))]

    # ---- I/O ----
    F8M = F8 if MM_FP8 else F16
    xg = nc.dram_tensor("xg", [NTOT, 128], F16, kind="ExternalInput")
    xT = nc.dram_tensor("xT", [128, NPC], F16, kind="ExternalInput")
    idx_d = nc.dram_tensor("idx", [128, NG * 8], I16, kind="ExternalInput")
    mm_d = nc.dram_tensor("mmoh", [128, NG * 128], F8M, kind="ExternalInput")
    inv_d = nc.dram_tensor("invdeg", [NW, 128], F32, kind="ExternalInput")
    nreal_d = nc.dram_tensor("nreal", [1, NB * NCH], mybir.dt.int32,
                             kind="ExternalInput")
    wnames = ["wl1T", "wr1T", "res1T", "wl2T", "wr2T", "res2T",
              "wl3T", "wr3T", "ff1T", "ff2T", "clfT"]
    wd = {n: nc.dram_tensor(n, [128, 128], F16, kind="ExternalInput")
          for n in wnames}
    # packed per-partition params: col 0:b3,1:ff1b,2:ff2b,3:clfb,
    # 4:bn1g,5:bn1b,6:bn2g,7:bn2b,8:res1b,9:res2b
    par_d = nc.dram_tensor("par", [128, 10], F32, kind="ExternalInput")
    out_d = nc.dram_tensor("out", [NPC, 64], F32, kind="ExternalOutput")

    h_loc = [nc.dram_tensor(f"h_loc{i}", [NPC, 128], F16, kind="Internal")
             for i in range(2)]
    hTd = [nc.dram_tensor(f"hTd{i}", [128, NPC], F16, kind="Internal")
           for i in range(2)]
    h_full = [nc.dram_tensor(f"h_full{i}", [NTOT, 128], F16, kind="Internal",
                             addr_space="Shared") for i in range(2)]
    st_in = [nc.dram_tensor(f"st_in{i}", [128, 2], F32, kind="Internal")
             for i in range(2)]
    st_out = [nc.dram_tensor(f"st_out{i}", [128, 2], F32, kind="Internal",
                             addr_space="Shared") for i in range(2)]

    xe_groups_max = max(
        int(sum(G[bin_rank[w, c]] for w in cfg.block_windows(b)))
        for b in range(NB) for c in range(NCH)) or 1

    with tile.TileContext(nc) as tc:
        with (
            tc.tile_pool(name="persist", bufs=1) as persist,
            tc.tile_pool(name="xe_p", bufs=10) as xe_p,
            tc.tile_pool(name="x8_p", bufs=8) as x8_p,
            tc.tile_pool(name="m_p", bufs=10) as m_p,
            tc.tile_pool(name="sm", bufs=3) as sm,
            tc.tile_pool(name="smb", bufs=2) as smb,
            tc.tile_pool(name="hp_p", bufs=2) as hp_p,
            tc.tile_pool(name="agg_pp", bufs=2, space="PSUM") as agg_pp,
            tc.tile_pool(name="z_pp", bufs=2, space="PSUM") as z_pp,
            tc.tile_pool(name="r_pp", bufs=2, space="PSUM") as r_pp,
            tc.tile_pool(name="t_pp", bufs=2, space="PSUM") as t_pp,
        ):
            # ---- persistent loads ----
            idx_sb = persist.tile([128, NG * 8], I16)
            nc.sync.dma_start(out=idx_sb[:, :], in_=idx_d[:, :])
            nreal_sb = persist.tile([1, NB * NCH], mybir.dt.int32)
            nc.sync.dma_start(out=nreal_sb[:, :], in_=nreal_d[:, :])
            nreal_regs = [nc.gpsimd.alloc_register(f"nreal_r{i}")
                          for i in range(8)]
            w_sb = {}
            for n in wnames:
                w_sb[n] = persist.tile([128, 128], F16, name=f"w_{n}")
                nc.sync.dma_start(out=w_sb[n][:, :], in_=wd[n][:, :])
            par_sb = persist.tile([128, 10], F32)
            nc.sync.dma_start(out=par_sb[:, :], in_=par_d[:, :])
            eps_sb = persist.tile([128, 1], F32)
            nc.vector.memset(eps_sb[:, :], EPS)
            # bn affine params per layer: cols 0=scale,1=bias,2=mean,3=tmp,4=tmp2
            bnp_sb = persist.tile([128, 6], F32)

            # zero-init the xe pool buffers: slots trimmed by trailing -1
            # indices are still read by matmuls (times a 0.0 one-hot column),
            # and uninitialized SBUF bits could decode as NaN/Inf (0*NaN=NaN).
            for _ in range(10):
                t0_ = xe_p.tile([128, xe_groups_max, 128], F16, name="xe",
                                tag="xe",
                                padded_shape=[128, xe_groups_max, 128])
                nc.vector.memset(t0_[:, :, :], 0.0)

            z_sb = persist.tile([128, NPC], F16)
            stats_sb = persist.tile([128, 2], F32)
            stats_in_sb = persist.tile([128, 2], F32)
            sums_sb = persist.tile([128, NB], F32)
            sqs_sb = persist.tile([128, NB], F32)

            def scatter_pass(layer, table_ap, hprev_d):
                """Pass A: aggregate + dense matmuls -> z_sb (+ stats).

                hprev_d: DRAM [128, NPC] fp16 (feature-major prev acts)."""
                for b in range(NB):
                    wins = list(cfg.block_windows(b))
                    nwin = len(wins)
                    agg = agg_pp.tile([128, nwin * 128], F32, name="agg",
                                      tag="agg", padded_shape=[128, 512])
                    # gather + cast + one-hot load for all chunks of this
                    # block; issue order rotates per block so the first
                    # (pacing) gather staggers across the 4 Q7 pairs
                    chunk_tiles = [None] * NCH
                    for c in [(b + j) % NCH for j in range(NCH)]:
                        gb0 = int(g_off[bin_rank[wins[0], c]])
                        ng = int(sum(G[bin_rank[w, c]] for w in wins))
                        if ng == 0:
                            continue
                        nidx = ng * 128
                        xe = xe_p.tile([128, ng, 128], F16, name="xe", tag="xe",
                                       padded_shape=[128, xe_groups_max, 128])
                        ci = b * NCH + c
                        cnt = nreal_regs[ci % 8]
                        nc.gpsimd.reg_load(cnt, nreal_sb[0:1, ci:ci + 1])
                        nc.gpsimd.dma_gather(
                            out_ap=xe[:, :, :],
                            in_ap=table_ap[cfg.cstart[c]:
                                           cfg.cstart[c] + cfg.csize[c], :],
                            idxs_ap=idx_sb[:, gb0 * 8:(gb0 + ng) * 8],
                            num_idxs=nidx, num_idxs_reg=cnt, elem_size=128,
                            single_packet=False, queue_num=c % 4)
                        mm = m_p.tile([128, ng, 128], F8M, name="mm", tag="mm",
                                      padded_shape=[128, xe_groups_max, 128])
                        nc.sync.dma_start(
                            out=mm[:, :, :],
                            in_=mm_d[:, gb0 * 128:(gb0 + ng) * 128].rearrange(
                                "p (g f) -> p g f", g=ng))
                        if USE_FP8:
                            x8 = x8_p.tile([128, ng, 128], F8, name="x8",
                                           tag="x8",
                                           padded_shape=[128, xe_groups_max,
                                                         128])
                            nc.vector.tensor_copy(out=x8[:, :, :],
                                                  in_=xe[:, :, :])
                        else:
                            x8 = xe
                        chunk_tiles[c] = (x8, mm, gb0)
                    # matmuls: each window's accumulation contiguous so PSUM
                    # zero-region (full bank) never has two open groups
                    for wi, w in enumerate(wins):
                        entries = []  # (x8, mm, gi0, G)
                        for c in [(b + j) % NCH for j in range(NCH)]:
                            if chunk_tiles[c] is None:
                                continue
                            gwc = int(G[bin_rank[w, c]])
                            if gwc == 0:
                                continue
                            x8, mm, gb0 = chunk_tiles[c]
                            gi0 = int(g_off[bin_rank[w, c]]) - gb0
                            entries.append((x8, mm, gi0, gwc))
                        if not entries:
                            continue
                        # (tile, g, span): span 2 => fp8 DoubleRow pair
                        mms = []
                        for x8, mm, gi0, gwc in entries:
                            g = 0
                            while g < gwc:
                                span = 2 if (USE_FP8 and g + 1 < gwc) else 1
                                mms.append((x8, mm, gi0 + g, span))
                                g += span
                        for j, (x8, mm, g0, span) in enumerate(mms):
                            if span == 2:
                                nc.tensor.matmul(
                                    agg[:, ts(wi, 128)],
                                    lhsT=x8[:, g0:g0 + 2, :],
                                    rhs=mm[:, g0:g0 + 2, :],
                                    start=(j == 0), stop=(j == len(mms) - 1),
                                    perf_mode=mybir.MatmulPerfMode.DoubleRow)
                            else:
                                nc.tensor.matmul(
                                    agg[:, ts(wi, 128)],
                                    lhsT=x8[:, g0, :],
                                    rhs=mm[:, g0, :],
                                    start=(j == 0), stop=(j == len(mms) - 1))
                    # evict: mean_T = agg * invdeg (broadcast over partitions)
                    invB = smb.tile([128, nwin * 128], F32, name="invB",
                                    tag="invB", padded_shape=[128, 512])
                    inv_ap = bass.AP(tensor=inv_d, offset=wins[0] * 128,
                                     ap=[[0, 128], [1, nwin * 128]])
                    nc.sync.dma_start(out=invB[:, :], in_=inv_ap)
                    mean = sm.tile([128, nwin * 128], F16, name="mean",
                                   tag="mean", padded_shape=[128, 512])
                    empty = [wi for wi, w in enumerate(wins)
                             if all(G[bin_rank[w, c]] == 0 for c in range(NCH))]
                    if len(empty) == nwin:
                        nc.vector.memset(mean[:, :], 0.0)
                    else:
                        nc.vector.tensor_tensor(out=mean[:, :], in0=agg[:, :],
                                                in1=invB[:, :], op=ALU.mult)
                        for wi in empty:
                            nc.vector.memset(mean[:, ts(wi, 128)], 0.0)
                    # dense: z = WlT.T@mean + WrT.T@hprev
                    wl, wr = (("wl1T", "wr1T"), ("wl2T", "wr2T"),
                              ("wl3T", "wr3T"))[layer]
                    hp = hp_p.tile([128, nwin * 128], F16, name="hp",
                                   tag="hp", padded_shape=[128, 512])
                    nc.sync.dma_start(
                        out=hp[:, :],
                        in_=hprev_d[:, b * 512:b * 512 + nwin * 128])
                    zp = z_pp.tile([128, nwin * 128], F32, name="zp", tag="zp",
                                   padded_shape=[128, 512])
                    nc.tensor.matmul(zp[:, :], lhsT=w_sb[wl][:, :],
                                     rhs=mean[:, :], start=True, stop=False)
                    nc.tensor.matmul(zp[:, :], lhsT=w_sb[wr][:, :],
                                     rhs=hp[:, :], start=False, stop=True)
                    if layer < 2:
                        # evict to fp16 z, accumulate sum + sumsq partials
                        nc.scalar.activation(z_sb[:, b * 512:b * 512 + nwin * 128],
                                             zp[:, :], ACTF.Copy,
                                             accum_out=sums_sb[:, b:b + 1])
                        sq = sm.tile([128, nwin * 128], F16, name="sq",
                                     tag="sq", padded_shape=[128, 512])
                        nc.scalar.activation(sq[:, :], zp[:, :], ACTF.Square,
                                             accum_out=sqs_sb[:, b:b + 1])
                    else:
                        # layer 3: z + b3 directly, no BN; head fused per block
                        nc.scalar.activation(z_sb[:, b * 512:b * 512 + nwin * 128],
                                             zp[:, :], ACTF.Identity,
                                             bias=par_sb[:, 0:1], scale=1.0)
                        head_block(b)

            def bn_params(layer):
                """AllReduce stats; compute scale/bias cols in bnp_sb."""
                si, so = st_in[layer], st_out[layer]
                nc.vector.reduce_sum(stats_in_sb[:, 0:1], sums_sb[:, :],
                                     axis=AX.X)
                nc.vector.reduce_sum(stats_in_sb[:, 1:2], sqs_sb[:, :],
                                     axis=AX.X)
                nc.sync.dma_start(out=si[:, :], in_=stats_in_sb[:, :])
                nc.gpsimd.collective_compute(
                    "AllReduce", ALU.add, replica_groups=rg,
                    ins=[si[:, :]], outs=[so[:, :]])
                nc.sync.dma_start(out=stats_sb[:, :], in_=so[:, :])
                g_ap = par_sb[:, 4 + 2 * layer:5 + 2 * layer]
                beta_ap = par_sb[:, 5 + 2 * layer:6 + 2 * layer]
                mean_ap = bnp_sb[:, 2:3]
                tmp_ap = bnp_sb[:, 3:4]
                tmp2_ap = bnp_sb[:, 4:5]
                # mean = s0/N ; ez2 = s1/N
                nc.scalar.activation(mean_ap, stats_sb[:, 0:1], ACTF.Copy,
                                     scale=1.0 / N_real)
                nc.scalar.activation(tmp_ap, stats_sb[:, 1:2], ACTF.Copy,
                                     scale=1.0 / N_real)
                # var = ez2 - mean^2
                nc.vector.tensor_tensor(out=tmp2_ap, in0=mean_ap, in1=mean_ap,
                                        op=ALU.mult)
                nc.vector.tensor_tensor(out=tmp_ap, in0=tmp_ap, in1=tmp2_ap,
                                        op=ALU.subtract)
                # rstd = 1/sqrt(var + eps)
                nc.scalar.activation(tmp_ap, tmp_ap, ACTF.Sqrt,
                                     bias=eps_sb[:, 0:1])
                nc.vector.reciprocal(tmp_ap, tmp_ap)
                # scale = rstd*g ; bias = beta - mean*scale
                nc.vector.tensor_tensor(out=bnp_sb[:, 0:1], in0=tmp_ap,
                                        in1=g_ap, op=ALU.mult)
                nc.vector.tensor_tensor(out=tmp2_ap, in0=mean_ap,
                                        in1=bnp_sb[:, 0:1], op=ALU.mult)
                nc.vector.tensor_tensor(out=bnp_sb[:, 1:2], in0=beta_ap,
                                        in1=tmp2_ap, op=ALU.subtract)

            def pass_b(layer, hprev_d, hnew_d, hloc, ag_out):
                """relu(bn(z)) + res -> hnew (fp16, DRAM); transpose+write h_loc.

                Issues the quarter-q AllGather (hloc rows -> ag_out chunk q)
                as soon as the blocks covering quarter q are written."""
                resw = ("res1T", "res2T")[layer]
                for b in range(NB):
                    wins = list(cfg.block_windows(b))
                    nwin = len(wins)
                    bs = b * 512
                    hp = hp_p.tile([128, nwin * 128], F16, name="hpb",
                                   tag="hp", padded_shape=[128, 512])
                    nc.sync.dma_start(out=hp[:, :],
                                      in_=hprev_d[:, bs:bs + nwin * 128])
                    rp = r_pp.tile([128, nwin * 128], F32, name="rp", tag="rp",
                                   padded_shape=[128, 512])
                    nc.tensor.matmul(rp[:, :], lhsT=w_sb[resw][:, :],
                                     rhs=hp[:, :], start=True, stop=True)
                    hbuf = sm.tile([128, nwin * 128], F32, name="hbuf",
                                   tag="hbuf", padded_shape=[128, 512])
                    nc.scalar.activation(hbuf[:, :], z_sb[:, bs:bs + nwin * 128],
                                         ACTF.Relu, bias=bnp_sb[:, 1:2],
                                         scale=bnp_sb[:, 0:1])
                    hf = sm.tile([128, nwin * 128], F32, name="hf", tag="hf",
                                 padded_shape=[128, 512])
                    # hf = (hbuf + res_bias) + res_matmul
                    nc.vector.scalar_tensor_tensor(
                        out=hf[:, :], in0=hbuf[:, :],
                        scalar=par_sb[:, 8 + layer:9 + layer],
                        in1=rp[:, :], op0=ALU.add, op1=ALU.add)
                    if b == NB - 1 and NPC > cfg.PCR:
                        # zero pad-node columns (keeps next-layer stats clean)
                        pstart = cfg.PCR - bs
                        nc.vector.memset(hf[:, pstart:nwin * 128], 0.0)
                    h16 = sm.tile([128, nwin * 128], F16, name="h16",
                                  tag="h16", padded_shape=[128, 512])
                    nc.vector.tensor_copy(out=h16[:, :], in_=hf[:, :])
                    nc.sync.dma_start(out=hnew_d[:, bs:bs + nwin * 128],
                                      in_=h16[:, :])
                    # transpose to node-major and store
                    tp = t_pp.tile([128, nwin * 128], F32, name="tp", tag="tp",
                                   padded_shape=[128, 512])
                    for wi in range(nwin):
                        nc.tensor.transpose(tp[:, ts(wi, 128)],
                                            hf[:, ts(wi, 128)],
                                            iden_sb[:, :])
                    wb = sm.tile([128, nwin * 128], F16, name="wb", tag="wb",
                                 padded_shape=[128, 512])
                    nc.vector.tensor_copy(out=wb[:, :], in_=tp[:, :])
                    dst_ap = bass.AP(
                        tensor=hloc, offset=bs * 128,
                        ap=[[128, 128], [128 * 128, nwin], [1, 128]])
                    nc.sync.dma_start(out=dst_ap, in_=wb[:, :].rearrange(
                        "p (w f) -> p w f", w=nwin))
                    for q in range(NCH):
                        if cfg.q_done_block[q] == b:
                            nc.gpsimd.collective_compute(
                                "AllGather", ALU.bypass, replica_groups=rg,
                                ins=[hloc[cfg.qsrow[q]:
                                          cfg.qsrow[q] + cfg.qrows[q], :]],
                                outs=[ag_out[cfg.cstart[q]:
                                             cfg.cstart[q] + cfg.csize[q], :]])

            def head_block(b):
                """relu(ff1@z+b) -> ff2 -> clf; transpose; write out (one block)."""
                if True:
                    wins = list(cfg.block_windows(b))
                    nwin = len(wins)
                    bs = b * 512
                    q1p = z_pp.tile([128, nwin * 128], F32, name="q1p",
                                    tag="zp", padded_shape=[128, 512])
                    nc.tensor.matmul(q1p[:, :], lhsT=w_sb["ff1T"][:, :],
                                     rhs=z_sb[:, bs:bs + nwin * 128],
                                     start=True, stop=True)
                    q1 = sm.tile([128, nwin * 128], F16, name="q1", tag="mean",
                                 padded_shape=[128, 512])
                    nc.scalar.activation(q1[:, :], q1p[:, :], ACTF.Relu,
                                         bias=par_sb[:, 1:2])
                    q2p = r_pp.tile([128, nwin * 128], F32, name="q2p",
                                    tag="rp", padded_shape=[128, 512])
                    nc.tensor.matmul(q2p[:, :], lhsT=w_sb["ff2T"][:, :],
                                     rhs=q1[:, :], start=True, stop=True)
                    q2 = sm.tile([128, nwin * 128], F16, name="q2", tag="sq",
                                 padded_shape=[128, 512])
                    nc.scalar.activation(q2[:, :], q2p[:, :], ACTF.Identity,
                                         bias=par_sb[:, 2:3])
                    op_ = z_pp.tile([128, nwin * 128], F32, name="op_",
                                    tag="zp", padded_shape=[128, 512])
                    nc.tensor.matmul(op_[:, :], lhsT=w_sb["clfT"][:, :],
                                     rhs=q2[:, :], start=True, stop=True)
                    ob = sm.tile([128, nwin * 128], F32, name="ob", tag="hbuf",
                                 padded_shape=[128, 512])
                    nc.scalar.activation(ob[:, :], op_[:, :], ACTF.Identity,
                                         bias=par_sb[:, 3:4])
                    tp = t_pp.tile([128, nwin * 128], F32, name="tp2", tag="tp",
                                   padded_shape=[128, 512])
                    for wi in range(nwin):
                        nc.tensor.transpose(tp[:, ts(wi, 128)],
                                            ob[:, ts(wi, 128)],
                                            iden_sb[:, :])
                    owb = sm.tile([128, nwin * 64], F32, name="owb", tag="hf",
                                  padded_shape=[128, 512])
                    for wi in range(nwin):
                        nc.vector.tensor_copy(out=owb[:, ts(wi, 64)],
                                              in_=tp[:, wi * 128:wi * 128 + 64])
                    dst_ap = bass.AP(
                        tensor=out_d, offset=bs * 64,
                        ap=[[64, 128], [128 * 64, nwin], [1, 64]])
                    nc.sync.dma_start(out=dst_ap, in_=owb[:, :].rearrange(
                        "p (w f) -> p w f", w=nwin))

            from concourse.masks import make_identity
            iden_sb = persist.tile([128, 128], F32)
            make_identity(nc, iden_sb[:, :])

            # ================= layer 1 =================
            scatter_pass(0, xg[:, :], xT[:, :])
            bn_params(0)
            pass_b(0, xT[:, :], hTd[0][:, :], h_loc[0], h_full[0])
            # ================= layer 2 =================
            scatter_pass(1, h_full[0][:, :], hTd[0][:, :])
            bn_params(1)
            pass_b(1, hTd[0][:, :], hTd[1][:, :], h_loc[1], h_full[1])
            # ================= layer 3 + head (fused) ==
            scatter_pass(2, h_full[1][:, :], hTd[1][:, :])

            if os.environ.get("BASS_GNN_DEBUG"):
                dbg = nc.dram_tensor("dbg_h1", [NTOT, 128], F16,
                                     kind="ExternalOutput")
                with tc.tile_pool(name="dbgp", bufs=2) as dbgp:
                    for i in range(NTOT // 128):
                        dt_ = dbgp.tile([128, 128], F16, name="dt_", tag="dt")
                        nc.sync.dma_start(
                            out=dt_[:, :],
                            in_=h_full[0][i * 128:(i + 1) * 128, :])
                        nc.sync.dma_start(
                            out=dbg[i * 128:(i + 1) * 128, :], in_=dt_[:, :])

    nc.compile()
    return nc


# ---------------- host side ----------------

_CACHE = {}


def _prepare(inputs):
    x = np.asarray(inputs["x"], dtype=np.float32)
    ei = np.asarray(inputs["edge_index"])
    cfg = Cfg(n_real=x.shape[0], e=ei.shape[1])
    plan = plan_edges(cfg, ei)

    key = (cfg.N, cfg.E, MODE, tuple(plan["G"].tolist()))
    if key not in _CACHE:
        _CACHE.clear()
        _CACHE[key] = build_kernel(cfg, plan["G"], plan["g_off"], plan["NG"],
                                   plan["bin_rank"])
    nc = _CACHE[key]

    # node-major fp16 gather table (padded, quarter-major chunk layout)
    xg = np.zeros((cfg.NTOT, 128), dtype=np.float16)
    for k in range(cfg.NC):
        for q in range(cfg.NCHUNK):
            nreal = min(cfg.qrows[q], max(0, cfg.PCR - cfg.qsrow[q]))
            if nreal <= 0:
                continue
            d0 = cfg.cstart[q] + k * cfg.qrows[q]
            s0 = k * cfg.PCR + cfg.qsrow[q]
            xg[d0:d0 + nreal] = x[s0:s0 + nreal].astype(np.float16)

    def w16(a):
        return np.ascontiguousarray(np.asarray(a, np.float32).T).astype(np.float16)

    clfT = np.zeros((128, 128), dtype=np.float16)
    clfT[:, :64] = w16(inputs["clf_w"])
    par = np.zeros((128, 10), dtype=np.float32)
    par[:, 0] = np.asarray(inputs["s3_b"], np.float32)
    par[:, 1] = np.asarray(inputs["ff1_b"], np.float32)
    par[:, 2] = np.asarray(inputs["ff2_b"], np.float32)
    par[:64, 3] = np.asarray(inputs["clf_b"], np.float32)
    par[:, 4] = np.asarray(inputs["bn1_g"], np.float32)
    par[:, 5] = np.asarray(inputs["bn1_b"], np.float32)
    par[:, 6] = np.asarray(inputs["bn2_g"], np.float32)
    par[:, 7] = np.asarray(inputs["bn2_b"], np.float32)
    par[:, 8] = np.asarray(inputs["res1_b"], np.float32)
    par[:, 9] = np.asarray(inputs["res2_b"], np.float32)

    weights = dict(
        wl1T=w16(inputs["s1_wl"]), wr1T=w16(inputs["s1_wr"]),
        res1T=w16(inputs["res1_w"]),
        wl2T=w16(inputs["s2_wl"]), wr2T=w16(inputs["s2_wr"]),
        res2T=w16(inputs["res2_w"]),
        wl3T=w16(inputs["s3_wl"]), wr3T=w16(inputs["s3_wr"]),
        ff1T=w16(inputs["ff1_w"]), ff2T=w16(inputs["ff2_w"]), clfT=clfT)

    in_maps = []
    for k in range(cfg.NC):
        xslice = np.zeros((128, cfg.NPC), dtype=np.float16)
        xslice[:, :cfg.PCR] = x[k * cfg.PCR:(k + 1) * cfg.PCR].T.astype(np.float16)
        im = dict(xg=xg, xT=xslice, idx=plan["idx"][k], mmoh=plan["mmoh"][k],
                  invdeg=plan["inv"][k], nreal=plan["nreal"][k], par=par,
                  **weights)
        in_maps.append(im)
    return cfg, nc, in_maps


_LAST_EXEC_NS = None
_LAST_RESULTS = None


def _ensure_axon_hooks():
    """The image's antenv lacks axon_hooks; shim it so NTFF tracing works."""
    import sys
    import types
    try:
        import antenv.axon_hooks  # noqa: F401
        return
    except ImportError:
        pass
    import antenv
    mod = types.ModuleType("antenv.axon_hooks")
    mod._hook = None

    def set_axon_ntff_profile_hook(h):
        mod._hook = h

    def get_axon_ntff_profile_hook():
        return mod._hook

    mod.set_axon_ntff_profile_hook = set_axon_ntff_profile_hook
    mod.get_axon_ntff_profile_hook = get_axon_ntff_profile_hook
    sys.modules["antenv.axon_hooks"] = mod
    antenv.axon_hooks = mod
    try:
        from trn_agent_boot.trn_boot import _ntff_profile_via_ctypes
        so = "/opt/axon/libaxon_pjrt.so"
        if os.path.exists(so):
            mod._hook = _ntff_profile_via_ctypes(so)
    except Exception as e:  # pragma: no cover
        print("ntff hook shim failed:", e)


def kernel(**inputs) -> np.ndarray:
    global _LAST_EXEC_NS, _LAST_RESULTS
    cfg, nc, in_maps = _prepare(inputs)
    trace = bool(int(os.environ.get("BASS_GNN_TRACE", "0")))
    if trace:
        _ensure_axon_hooks()
    res = bass_utils.run_bass_kernel_spmd(
        nc, in_maps, core_ids=list(range(cfg.NC)), trace=trace)
    _LAST_EXEC_NS = res.exec_time_ns
    _LAST_RESULTS = res
    out = np.empty((cfg.N, 64), dtype=np.float32)
    for k in range(cfg.NC):
        out[k * cfg.PCR:(k + 1) * cfg.PCR] = res.results[k]["out"][:cfg.PCR]
    return out



# revision 46
# speedup vs baseline: 1.0290x; 1.0290x over previous
"""Trainium2 Bass kernel for EnhancedSAGEModel (3x SAGEConv mean-aggr + BN + FFN head).

Sharding: nodes across 8 cores (12500 real + 44 pad slots each); edges
partitioned by destination; per-layer dma_gather of source rows from a
replicated node-feature table (fp16, quarter-major chunk layout), scatter-add
via one-hot matmul into PSUM in feature-major layout; chunked AllGather of
activations between layers (quarter q issues as soon as pass_b wrote it);
AllReduce for BatchNorm batch statistics.

Perf notes (6.56ms -> ~2.7ms on 8 axon trn2 cores):
- gathers spread over all 4 SWDGE queues (num_swdge_queues=4, queue_num=c%4):
  each queue runs on its own Q7 core pair; descriptor generation is the
  kernel's pacing resource (~8ns/idx per pair).
- the one-hot scatter matrix is precomputed on the host in fp8e4 (0/1 exact)
  and DMA-loaded via HWDGE; matmuls run mixed fp16(lhsT=xe) x fp8(rhs).
- per-(block,chunk) trailing pad indices are -1 with the per-core real count
  passed via num_idxs_reg (rotating registers): the gather ucode skips them.
- xe pool buffers are zero-initialized once (trimmed slots are still read by
  matmuls against 0.0 one-hot columns; stale NaN * 0 would poison PSUM).
- layer-3 FFN+classifier head is fused per block into the scatter pass.

Self-contained: hardcodes the problem shapes from spec
(x:[100000,128] f32, edge_index:[2,1600000] i64, weights 128x128 etc.).
"""

import math
import os
import numpy as np
import ml_dtypes

import concourse.bass as bass
import concourse.bacc as bacc
import concourse.mybir as mybir
import concourse.tile as tile
from concourse.bass import ts
from concourse import bass_utils

F16 = mybir.dt.float16
F32 = mybir.dt.float32
F8 = mybir.dt.float8e4
NP_F8 = ml_dtypes.float8_e4m3
I16 = mybir.dt.int16

# mm8: one-hot in fp8 (exact), xe fp16, plain matmuls (default)
# dr8: one-hot + xe in fp8, DoubleRow paired matmuls (fast, ~1.4e-2 rel err)
# fp16: everything fp16
MODE = os.environ.get("BASS_GNN_MODE", "mm8")
assert MODE in ("mm8", "dr8", "fp16")
USE_FP8 = MODE == "dr8"          # xe cast + DoubleRow
MM_FP8 = MODE in ("mm8", "dr8")  # one-hot dtype

AX = mybir.AxisListType
ALU = mybir.AluOpType
ACTF = mybir.ActivationFunctionType

EPS = 1e-5


class Cfg:
    """Quarter-major table layout: global table row of node (core k, local i)
    with i in quarter q (by window) is cstart[q] + k*qrows[q] + (i - qsrow[q]).
    Chunk c == quarter c, so the next-layer gather of chunk c only depends on
    the AllGather of quarter c (enables chunked AG overlap with pass_b)."""

    def __init__(self, n_real, e, n_cores=8, d=128, c_out=64):
        self.N = n_real
        self.E = e
        self.NC = n_cores
        self.D = d
        self.C = c_out
        assert n_real % n_cores == 0
        self.PCR = n_real // n_cores              # real nodes per core
        self.NW = math.ceil(self.PCR / 128)       # windows per core (128 dst each)
        self.NPC = self.NW * 128                  # padded nodes per core
        self.NTOT = self.NPC * n_cores            # padded global nodes
        self.NCHUNK = 4
        base, rem = divmod(self.NW, self.NCHUNK)
        self.QW = [base + (1 if i < rem else 0) for i in range(self.NCHUNK)]
        assert self.QW[0] * 128 * n_cores <= 32767
        self.QWS = np.concatenate([[0], np.cumsum(self.QW)])  # window starts
        self.qrows = [w * 128 for w in self.QW]               # rows per quarter
        self.qsrow = [int(s) * 128 for s in self.QWS[:-1]]    # local row starts
        self.csize = [n_cores * r for r in self.qrows]        # chunk sizes
        self.cstart = [0]
        for s in self.csize[:-1]:
            self.cstart.append(self.cstart[-1] + s)
        for cs in self.csize:
            assert cs <= 32767, "int16 gather index limit"
        self.NB = math.ceil(self.NW / 4)          # dense blocks (<=4 windows each)
        # pass_b block after which quarter q's h_loc rows are all written
        self.q_done_block = [math.ceil(int(self.QWS[q + 1]) / 4) - 1
                             for q in range(self.NCHUNK)]

    def block_windows(self, b):
        return range(4 * b, min(4 * b + 4, self.NW))


def plan_edges(cfg, edge_index):
    """Bin edges per core by (dense-block, chunk, window); pad each bin to a
    multiple of 128 slots. Returns per-core plan arrays + shared structure."""
    src = np.asarray(edge_index[0], dtype=np.int64)
    dst = np.asarray(edge_index[1], dtype=np.int64)
    NC, PCR, NPC = cfg.NC, cfg.PCR, cfg.NPC
    NW, NCH = cfg.NW, cfg.NCHUNK

    deg = np.bincount(dst, minlength=cfg.N).astype(np.float32)
    invdeg_full = (1.0 / np.maximum(deg, 1.0)).astype(np.float32)

    core = dst // PCR
    ld_all = dst % PCR                       # local dst in [0, PCR)
    w_all = ld_all // 128
    ldw_all = (ld_all % 128).astype(np.float16)
    # quarter-major chunk mapping of sources
    k_src = src // PCR
    i_src = src % PCR
    c_all = np.searchsorted(cfg.QWS[1:], i_src // 128, side="right")
    qrows_a = np.asarray(cfg.qrows, dtype=np.int64)
    qsrow_a = np.asarray(cfg.qsrow, dtype=np.int64)
    lsrc_all = (k_src * qrows_a[c_all] + (i_src - qsrow_a[c_all])).astype(
        np.int16)

    # bin linear order: for block: for chunk: for window-in-block
    # bin_rank(w, c) -> position in that order
    bin_rank = np.empty((NW, NCH), dtype=np.int64)
    r = 0
    order = []
    for b in range(cfg.NB):
        for c in range(NCH):
            for w in cfg.block_windows(b):
                bin_rank[w, c] = r
                order.append((w, c))
                r += 1
    NBINS = r

    counts = np.zeros((NC, NBINS), dtype=np.int64)
    binid = bin_rank[w_all, c_all]
    for k in range(NC):
        m = core == k
        counts[k] = np.bincount(binid[m], minlength=NBINS)

    G = np.maximum(np.ceil(counts.max(axis=0) / 128.0).astype(np.int64), 0)
    g_off = np.concatenate([[0], np.cumsum(G)])  # group offset per bin
    NG = int(g_off[-1])

    mm_np = NP_F8 if MM_FP8 else np.float16
    idx_plans, mm_plans, inv_plans, nreal_plans = [], [], [], []
    for k in range(NC):
        m = core == k
        bk = binid[m]
        lsrc_k = lsrc_all[m]
        ldw_k = ldw_all[m]
        ordk = np.argsort(bk, kind="stable")
        bk = bk[ordk]
        lsrc_k = lsrc_k[ordk]
        ldw_k = ldw_k[ordk].astype(np.int64)
        starts = np.concatenate([[0], np.cumsum(counts[k])])
        rank = np.arange(len(bk)) - starts[bk]

        idx_arr = np.zeros((128, NG * 8), dtype=np.int16)
        # Trailing pad of the LAST nonempty bin of each (block, chunk) gather
        # call: mark idx=-1 and record the per-core real count (passed as
        # num_idxs_reg) so the ucode skips those descriptors at runtime.
        nreal = np.zeros(cfg.NB * NCH, dtype=np.int32)
        for b in range(cfg.NB):
            wins_b = list(cfg.block_windows(b))
            for c in range(NCH):
                gb0 = g_off[bin_rank[wins_b[0], c]]
                ng_call = int(sum(G[bin_rank[w, c]] for w in wins_b))
                if ng_call == 0:
                    continue
                wl = max(w for w in wins_b if G[bin_rank[w, c]] > 0)
                bin_ = bin_rank[wl, c]
                gb, ge = g_off[bin_], g_off[bin_ + 1]
                pad_r = np.arange(counts[k][bin_], (ge - gb) * 128)
                idx_arr[pad_r % 16, gb * 8 + pad_r // 16] = -1
                nreal[b * NCH + c] = (gb - gb0) * 128 + counts[k][bin_]
        col = g_off[bk] * 8 + rank // 16
        idx_arr[rank % 16, col] = lsrc_k
        # HW ucode: each of the 8 Q7 cores reads its own 16-partition group
        idx_arr = np.tile(idx_arr[:16], (8, 1))
        # host-precomputed one-hot scatter matrix: slot (g*128+p) -> dst d
        slot = g_off[bk] * 128 + rank
        mm_arr = np.zeros((128, NG * 128), dtype=mm_np)
        mm_arr[slot % 128, (slot // 128) * 128 + ldw_k] = 1.0

        inv = np.ones((NPC,), dtype=np.float32)
        inv[:PCR] = invdeg_full[k * PCR:(k + 1) * PCR]
        inv_plans.append(inv.reshape(NW, 128))
        idx_plans.append(idx_arr)
        mm_plans.append(mm_arr)
        nreal_plans.append(nreal.reshape(1, -1))

    return dict(G=G, g_off=g_off, NG=NG, bin_rank=bin_rank,
                idx=idx_plans, mmoh=mm_plans, inv=inv_plans, nreal=nreal_plans)


def build_kernel(cfg, G, g_off, NG, bin_rank):
    """Build the 8-core SPMD Bass program. Structure (G table) is compile-time."""
    NW, NCH, NPC, NTOT, NB = cfg.NW, cfg.NCHUNK, cfg.NPC, cfg.NTOT, cfg.NB
    N_real = cfg.N

    nc = bacc.Bacc("TRN2", target_bir_lowering=False, debug=False,
                   num_devices=cfg.NC, num_swdge_queues=4)
    rg = [list(range(cfg.NC))]

    # ---- I/O ----
    F8M = F8 if MM_FP8 else F16
    xg = nc.dram_tensor("xg", [NTOT, 128], F16, kind="ExternalInput")
    xT = nc.dram_tensor("xT", [128, NPC], F16, kind="ExternalInput")
    idx_d = nc.dram_tensor("idx", [128, NG * 8], I16, kind="ExternalInput")
    mm_d = nc.dram_tensor("mmoh", [128, NG * 128], F8M, kind="ExternalInput")
    inv_d = nc.dram_tensor("invdeg", [NW, 128], F32, kind="ExternalInput")
    nreal_d = nc.dram_tensor("nreal", [1, NB * NCH], mybir.dt.int32,
                             kind="ExternalInput")
    wnames = ["wl1T", "wr1T", "res1T", "wl2T", "wr2T", "res2T",
              "wl3T", "wr3T", "ff1T", "ff2T", "clfT"]
    wd = {n: nc.dram_tensor(n, [128, 128], F16, kind="ExternalInput")
          for n in wnames}
    # packed per-partition params: col 0:b3,1:ff1b,2:ff2b,3:clfb,
    # 4:bn1g,5:bn1b,6:bn2g,7:bn2b,8:res1b,9:res2b
    par_d = nc.dram_tensor("par", [128, 10], F32, kind="ExternalInput")
    out_d = nc.dram_tensor("out", [NPC, 64], F32, kind="ExternalOutput")

    h_loc = [nc.dram_tensor(f"h_loc{i}", [NPC, 128], F16, kind="Internal")
             for i in range(2)]
    hTd = [nc.dram_tensor(f"hTd{i}", [128, NPC], F16, kind="Internal")
           for i in range(2)]
    h_full = [nc.dram_tensor(f"h_full{i}", [NTOT, 128], F16, kind="Internal",
                             addr_space="Shared") for i in range(2)]
    st_in = [nc.dram_tensor(f"st_in{i}", [128, 2], F32, kind="Internal")
             for i in range(2)]
    st_out = [nc.dram_tensor(f"st_out{i}", [128, 2], F32, kind="Internal",
                             addr_space="Shared") for i in range(2)]

    xe_groups_max = max(
        int(sum(G[bin_rank[w, c]] for w in cfg.block_windows(b)))
        for b in range(NB) for c in range(NCH)) or 1

    with tile.TileContext(nc) as tc:
        with (
            tc.tile_pool(name="persist", bufs=1) as persist,
            tc.tile_pool(name="xe_p", bufs=10) as xe_p,
            tc.tile_pool(name="x8_p", bufs=8) as x8_p,
            tc.tile_pool(name="m_p", bufs=10) as m_p,
            tc.tile_pool(name="sm", bufs=3) as sm,
            tc.tile_pool(name="smb", bufs=2) as smb,
            tc.tile_pool(name="hp_p", bufs=2) as hp_p,
            tc.tile_pool(name="agg_pp", bufs=2, space="PSUM") as agg_pp,
            tc.tile_pool(name="z_pp", bufs=2, space="PSUM") as z_pp,
            tc.tile_pool(name="r_pp", bufs=2, space="PSUM") as r_pp,
            tc.tile_pool(name="t_pp", bufs=2, space="PSUM") as t_pp,
        ):
            # ---- persistent loads ----
            idx_sb = persist.tile([128, NG * 8], I16)
            nc.sync.dma_start(out=idx_sb[:, :], in_=idx_d[:, :])
            nreal_sb = persist.tile([1, NB * NCH], mybir.dt.int32)
            nc.sync.dma_start(out=nreal_sb[:, :], in_=nreal_d[:, :])
            nreal_regs = [nc.gpsimd.alloc_register(f"nreal_r{i}")
                          for i in range(8)]
            w_sb = {}
            for n in wnames:
                w_sb[n] = persist.tile([128, 128], F16, name=f"w_{n}")
                nc.sync.dma_start(out=w_sb[n][:, :], in_=wd[n][:, :])
            par_sb = persist.tile([128, 10], F32)
            nc.sync.dma_start(out=par_sb[:, :], in_=par_d[:, :])
            eps_sb = persist.tile([128, 1], F32)
            nc.vector.memset(eps_sb[:, :], EPS)
            # bn affine params per layer: cols 0=scale,1=bias,2=mean,3=tmp,4=tmp2
            bnp_sb = persist.tile([128, 6], F32)

            # zero-init the xe pool buffers: slots trimmed by trailing -1
            # indices are still read by matmuls (times a 0.0 one-hot column),
            # and uninitialized SBUF bits could decode as NaN/Inf (0*NaN=NaN).
            for _ in range(10):
                t0_ = xe_p.tile([128, xe_groups_max, 128], F16, name="xe",
                                tag="xe",
                                padded_shape=[128, xe_groups_max, 128])
                nc.vector.memset(t0_[:, :, :], 0.0)

            z_sb = persist.tile([128, NPC], F16)
            stats_sb = persist.tile([128, 2], F32)
            stats_in_sb = persist.tile([128, 2], F32)
            sums_sb = persist.tile([128, NB], F32)
            sqs_sb = persist.tile([128, NB], F32)

            def scatter_pass(layer, table_ap, hprev_d):
                """Pass A: aggregate + dense matmuls -> z_sb (+ stats).

                hprev_d: DRAM [128, NPC] fp16 (feature-major prev acts)."""
                for b in range(NB):
                    wins = list(cfg.block_windows(b))
                    nwin = len(wins)
                    agg = agg_pp.tile([128, nwin * 128], F32, name="agg",
                                      tag="agg", padded_shape=[128, 512])
                    # gather + cast + one-hot load for all chunks of this
                    # block; issue order rotates per block so the first
                    # (pacing) gather staggers across the 4 Q7 pairs
                    chunk_tiles = [None] * NCH
                    for c in [(b + j) % NCH for j in range(NCH)]:
                        gb0 = int(g_off[bin_rank[wins[0], c]])
                        ng = int(sum(G[bin_rank[w, c]] for w in wins))
                        if ng == 0:
                            continue
                        nidx = ng * 128
                        xe = xe_p.tile([128, ng, 128], F16, name="xe", tag="xe",
                                       padded_shape=[128, xe_groups_max, 128])
                        ci = b * NCH + c
                        cnt = nreal_regs[ci % 8]
                        nc.gpsimd.reg_load(cnt, nreal_sb[0:1, ci:ci + 1])
                        nc.gpsimd.dma_gather(
                            out_ap=xe[:, :, :],
                            in_ap=table_ap[cfg.cstart[c]:
                                           cfg.cstart[c] + cfg.csize[c], :],
                            idxs_ap=idx_sb[:, gb0 * 8:(gb0 + ng) * 8],
                            num_idxs=nidx, num_idxs_reg=cnt, elem_size=128,
                            single_packet=False, queue_num=c % 4)
                        mm = m_p.tile([128, ng, 128], F8M, name="mm", tag="mm",
                                      padded_shape=[128, xe_groups_max, 128])
                        nc.sync.dma_start(
                            out=mm[:, :, :],
                            in_=mm_d[:, gb0 * 128:(gb0 + ng) * 128].rearrange(
                                "p (g f) -> p g f", g=ng))
                        if USE_FP8:
                            x8 = x8_p.tile([128, ng, 128], F8, name="x8",
                                           tag="x8",
                                           padded_shape=[128, xe_groups_max,
                                                         128])
                            nc.vector.tensor_copy(out=x8[:, :, :],
                                                  in_=xe[:, :, :])
                        else:
                            x8 = xe
                        chunk_tiles[c] = (x8, mm, gb0)
                    # matmuls: each window's accumulation contiguous so PSUM
                    # zero-region (full bank) never has two open groups
                    for wi, w in enumerate(wins):
                        entries = []  # (x8, mm, gi0, G)
                        for c in [(b + j) % NCH for j in range(NCH)]:
                            if chunk_tiles[c] is None:
                                continue
                            gwc = int(G[bin_rank[w, c]])
                            if gwc == 0:
                                continue
                            x8, mm, gb0 = chunk_tiles[c]
                            gi0 = int(g_off[bin_rank[w, c]]) - gb0
                            entries.append((x8, mm, gi0, gwc))
                        if not entries:
                            continue
                        # (tile, g, span): span 2 => fp8 DoubleRow pair
                        mms = []
                        for x8, mm, gi0, gwc in entries:
                            g = 0
                            while g < gwc:
                                span = 2 if (USE_FP8 and g + 1 < gwc) else 1
                                mms.append((x8, mm, gi0 + g, span))
                                g += span
                        for j, (x8, mm, g0, span) in enumerate(mms):
                            if span == 2:
                                nc.tensor.matmul(
                                    agg[:, ts(wi, 128)],
                                    lhsT=x8[:, g0:g0 + 2, :],
                                    rhs=mm[:, g0:g0 + 2, :],
                                    start=(j == 0), stop=(j == len(mms) - 1),
                                    perf_mode=mybir.MatmulPerfMode.DoubleRow)
                            else:
                                nc.tensor.matmul(
                                    agg[:, ts(wi, 128)],
                                    lhsT=x8[:, g0, :],
                                    rhs=mm[:, g0, :],
                                    start=(j == 0), stop=(j == len(mms) - 1))
                    # evict: mean_T = agg * invdeg (broadcast over partitions)
                    invB = smb.tile([128, nwin * 128], F32, name="invB",
                                    tag="invB", padded_shape=[128, 512])
                    inv_ap = bass.AP(tensor=inv_d, offset=wins[0] * 128,
                                     ap=[[0, 128], [1, nwin * 128]])
                    nc.sync.dma_start(out=invB[:, :], in_=inv_ap)
                    mean = sm.tile([128, nwin * 128], F16, name="mean",
                                   tag="mean", padded_shape=[128, 512])
                    empty = [wi for wi, w in enumerate(wins)
                             if all(G[bin_rank[w, c]] == 0 for c in range(NCH))]
                    if len(empty) == nwin:
                        nc.vector.memset(mean[:, :], 0.0)
                    else:
                        nc.vector.tensor_tensor(out=mean[:, :], in0=agg[:, :],
                                                in1=invB[:, :], op=ALU.mult)
                        for wi in empty:
                            nc.vector.memset(mean[:, ts(wi, 128)], 0.0)
                    # dense: z = WlT.T@mean + WrT.T@hprev
                    wl, wr = (("wl1T", "wr1T"), ("wl2T", "wr2T"),
                              ("wl3T", "wr3T"))[layer]
                    hp = hp_p.tile([128, nwin * 128], F16, name="hp",
                                   tag="hp", padded_shape=[128, 512])
                    nc.sync.dma_start(
                        out=hp[:, :],
                        in_=hprev_d[:, b * 512:b * 512 + nwin * 128])
                    zp = z_pp.tile([128, nwin * 128], F32, name="zp", tag="zp",
                                   padded_shape=[128, 512])
                    nc.tensor.matmul(zp[:, :], lhsT=w_sb[wl][:, :],
                                     rhs=mean[:, :], start=True, stop=False)
                    nc.tensor.matmul(zp[:, :], lhsT=w_sb[wr][:, :],
                                     rhs=hp[:, :], start=False, stop=True)
                    if layer < 2:
                        # evict to fp16 z, accumulate sum + sumsq partials
                        nc.scalar.activation(z_sb[:, b * 512:b * 512 + nwin * 128],
                                             zp[:, :], ACTF.Copy,
                                             accum_out=sums_sb[:, b:b + 1])
                        sq = sm.tile([128, nwin * 128], F16, name="sq",
                                     tag="sq", padded_shape=[128, 512])
                        nc.scalar.activation(sq[:, :], zp[:, :], ACTF.Square,
                                             accum_out=sqs_sb[:, b:b + 1])
                    else:
                        # layer 3: z + b3 directly, no BN; head fused per block
                        nc.scalar.activation(z_sb[:, b * 512:b * 512 + nwin * 128],
                                             zp[:, :], ACTF.Identity,
                                             bias=par_sb[:, 0:1], scale=1.0)
                        head_block(b)

            def bn_params(layer):
                """AllReduce stats; compute scale/bias cols in bnp_sb."""
                si, so = st_in[layer], st_out[layer]
                nc.vector.reduce_sum(stats_in_sb[:, 0:1], sums_sb[:, :],
                                     axis=AX.X)
                nc.vector.reduce_sum(stats_in_sb[:, 1:2], sqs_sb[:, :],
                                     axis=AX.X)
                nc.sync.dma_start(out=si[:, :], in_=stats_in_sb[:, :])
                nc.gpsimd.collective_compute(
                    "AllReduce", ALU.add, replica_groups=rg,
                    ins=[si[:, :]], outs=[so[:, :]])
                nc.sync.dma_start(out=stats_sb[:, :], in_=so[:, :])
                g_ap = par_sb[:, 4 + 2 * layer:5 + 2 * layer]
                beta_ap = par_sb[:, 5 + 2 * layer:6 + 2 * layer]
                mean_ap = bnp_sb[:, 2:3]
                tmp_ap = bnp_sb[:, 3:4]
                tmp2_ap = bnp_sb[:, 4:5]
                # mean = s0/N ; ez2 = s1/N
                nc.scalar.activation(mean_ap, stats_sb[:, 0:1], ACTF.Copy,
                                     scale=1.0 / N_real)
                nc.scalar.activation(tmp_ap, stats_sb[:, 1:2], ACTF.Copy,
                                     scale=1.0 / N_real)
                # var = ez2 - mean^2
                nc.vector.tensor_tensor(out=tmp2_ap, in0=mean_ap, in1=mean_ap,
                                        op=ALU.mult)
                nc.vector.tensor_tensor(out=tmp_ap, in0=tmp_ap, in1=tmp2_ap,
                                        op=ALU.subtract)
                # rstd = 1/sqrt(var + eps)
                nc.scalar.activation(tmp_ap, tmp_ap, ACTF.Sqrt,
                                     bias=eps_sb[:, 0:1])
                nc.vector.reciprocal(tmp_ap, tmp_ap)
                # scale = rstd*g ; bias = beta - mean*scale
                nc.vector.tensor_tensor(out=bnp_sb[:, 0:1], in0=tmp_ap,
                                        in1=g_ap, op=ALU.mult)
                nc.vector.tensor_tensor(out=tmp2_ap, in0=mean_ap,
                                        in1=bnp_sb[:, 0:1], op=ALU.mult)
                nc.vector.tensor_tensor(out=bnp_sb[:, 1:2], in0=beta_ap,
                                        in1=tmp2_ap, op=ALU.subtract)

            def pass_b(layer, hprev_d, hnew_d, hloc, ag_out):
                """relu(bn(z)) + res -> hnew (fp16, DRAM); transpose+write h_loc.

                Issues the quarter-q AllGather (hloc rows -> ag_out chunk q)
                as soon as the blocks covering quarter q are written."""
                resw = ("res1T", "res2T")[layer]
                for b in range(NB):
                    wins = list(cfg.block_windows(b))
                    nwin = len(wins)
                    bs = b * 512
                    hp = hp_p.tile([128, nwin * 128], F16, name="hpb",
                                   tag="hp", padded_shape=[128, 512])
                    nc.sync.dma_start(out=hp[:, :],
                                      in_=hprev_d[:, bs:bs + nwin * 128])
                    rp = r_pp.tile([128, nwin * 128], F32, name="rp", tag="rp",
                                   padded_shape=[128, 512])
                    nc.tensor.matmul(rp[:, :], lhsT=w_sb[resw][:, :],
                                     rhs=hp[:, :], start=True, stop=True)
                    hbuf = sm.tile([128, nwin * 128], F32, name="hbuf",
                                   tag="hbuf", padded_shape=[128, 512])
                    nc.scalar.activation(hbuf[:, :], z_sb[:, bs:bs + nwin * 128],
                                         ACTF.Relu, bias=bnp_sb[:, 1:2],
                                         scale=bnp_sb[:, 0:1])
                    hf = sm.tile([128, nwin * 128], F32, name="hf", tag="hf",
                                 padded_shape=[128, 512])
                    # hf = (hbuf + res_bias) + res_matmul
                    nc.vector.scalar_tensor_tensor(
                        out=hf[:, :], in0=hbuf[:, :],
                        scalar=par_sb[:, 8 + layer:9 + layer],
                        in1=rp[:, :], op0=ALU.add, op1=ALU.add)
                    if b == NB - 1 and NPC > cfg.PCR:
                        # zero pad-node columns (keeps next-layer stats clean)
                        pstart = cfg.PCR - bs
                        nc.vector.memset(hf[:, pstart:nwin * 128], 0.0)
                    h16 = sm.tile([128, nwin * 128], F16, name="h16",
                                  tag="h16", padded_shape=[128, 512])
                    nc.vector.tensor_copy(out=h16[:, :], in_=hf[:, :])
                    nc.sync.dma_start(out=hnew_d[:, bs:bs + nwin * 128],
                                      in_=h16[:, :])
                    # transpose to node-major and store
                    tp = t_pp.tile([128, nwin * 128], F32, name="tp", tag="tp",
                                   padded_shape=[128, 512])
                    for wi in range(nwin):
                        nc.tensor.transpose(tp[:, ts(wi, 128)],
                                            hf[:, ts(wi, 128)],
                                            iden_sb[:, :])
                    wb = sm.tile([128, nwin * 128], F16, name="wb", tag="wb",
                                 padded_shape=[128, 512])
                    nc.vector.tensor_copy(out=wb[:, :], in_=tp[:, :])
                    dst_ap = bass.AP(
                        tensor=hloc, offset=bs * 128,
                        ap=[[128, 128], [128 * 128, nwin], [1, 128]])
                    nc.sync.dma_start(out=dst_ap, in_=wb[:, :].rearrange(
                        "p (w f) -> p w f", w=nwin))
                    for q in range(NCH):
                        if cfg.q_done_block[q] == b:
                            nc.gpsimd.collective_compute(
                                "AllGather", ALU.bypass, replica_groups=rg,
                                ins=[hloc[cfg.qsrow[q]:
                                          cfg.qsrow[q] + cfg.qrows[q], :]],
                                outs=[ag_out[cfg.cstart[q]:
                                             cfg.cstart[q] + cfg.csize[q], :]])

            def head_block(b):
                """relu(ff1@z+b) -> ff2 -> clf; transpose; write out (one block)."""
                if True:
                    wins = list(cfg.block_windows(b))
                    nwin = len(wins)
                    bs = b * 512
                    q1p = z_pp.tile([128, nwin * 128], F32, name="q1p",
                                    tag="zp", padded_shape=[128, 512])
                    nc.tensor.matmul(q1p[:, :], lhsT=w_sb["ff1T"][:, :],
                                     rhs=z_sb[:, bs:bs + nwin * 128],
                                     start=True, stop=True)
                    q1 = sm.tile([128, nwin * 128], F16, name="q1", tag="mean",
                                 padded_shape=[128, 512])
                    nc.scalar.activation(q1[:, :], q1p[:, :], ACTF.Relu,
                                         bias=par_sb[:, 1:2])
                    q2p = r_pp.tile([128, nwin * 128], F32, name="q2p",
                                    tag="rp", padded_shape=[128, 512])
                    nc.tensor.matmul(q2p[:, :], lhsT=w_sb["ff2T"][:, :],
                                     rhs=q1[:, :], start=True, stop=True)
                    q2 = sm.tile([128, nwin * 128], F16, name="q2", tag="sq",
                                 padded_shape=[128, 512])
                    nc.scalar.activation(q2[:, :], q2p[:, :], ACTF.Identity,
                                         bias=par_sb[:, 2:3])
                    op_ = z_pp.tile([128, nwin * 128], F32, name="op_",
                                    tag="zp", padded_shape=[128, 512])
                    nc.tensor.matmul(op_[:, :], lhsT=w_sb["clfT"][:, :],
                                     rhs=q2[:, :], start=True, stop=True)
                    ob = sm.tile([128, nwin * 128], F32, name="ob", tag="hbuf",
                                 padded_shape=[128, 512])
                    nc.scalar.activation(ob[:, :], op_[:, :], ACTF.Identity,
                                         bias=par_sb[:, 3:4])
                    tp = t_pp.tile([128, nwin * 128], F32, name="tp2", tag="tp",
                                   padded_shape=[128, 512])
                    for wi in range(nwin):
                        nc.tensor.transpose(tp[:, ts(wi, 128)],
                                            ob[:, ts(wi, 128)],
                                            iden_sb[:, :])
                    owb = sm.tile([128, nwin * 64], F32, name="owb", tag="hf",
                                  padded_shape=[128, 512])
                    for wi in range(nwin):
                        nc.vector.tensor_copy(out=owb[:, ts(wi, 64)],
                                              in_=tp[:, wi * 128:wi * 128 + 64])
                    dst_ap = bass.AP(
                        tensor=out_d, offset=bs * 64,
                        ap=[[64, 128], [128 * 64, nwin], [1, 64]])
                    nc.sync.dma_start(out=dst_ap, in_=owb[:, :].rearrange(
                        "p (w f) -> p w f", w=nwin))

            from concourse.masks import make_identity
            iden_sb = persist.tile([128, 128], F32)
            make_identity(nc, iden_sb[:, :])

            # ================= layer 1 =================
            scatter_pass(0, xg[:, :], xT[:, :])
            bn_params(0)
            pass_b(0, xT[:, :], hTd[0][:, :], h_loc[0], h_full[0])
            # ================= layer 2 =================
            scatter_pass(1, h_full[0][:, :], hTd[0][:, :])
            bn_params(1)
            pass_b(1, hTd[0][:, :], hTd[1][:, :], h_loc[1], h_full[1])
            # ================= layer 3 + head (fused) ==
            scatter_pass(2, h_full[1][:, :], hTd[1][:, :])

            if os.environ.get("BASS_GNN_DEBUG"):
                dbg = nc.dram_tensor("dbg_h1", [NTOT, 128], F16,
                                     kind="ExternalOutput")
                with tc.tile_pool(name="dbgp", bufs=2) as dbgp:
                    for i in range(NTOT // 128):
                        dt_ = dbgp.tile([128, 128], F16, name="dt_", tag="dt")
                        nc.sync.dma_start(
                            out=dt_[:, :],
                            in_=h_full[0][i * 128:(i + 1) * 128, :])
                        nc.sync.dma_start(
                            out=dbg[i * 128:(i + 1) * 128, :], in_=dt_[:, :])

    nc.compile()
    return nc


# ---------------- host side ----------------

_CACHE = {}


def _prepare(inputs):
    x = np.asarray(inputs["x"], dtype=np.float32)
    ei = np.asarray(inputs["edge_index"])
    cfg = Cfg(n_real=x.shape[0], e=ei.shape[1])
    plan = plan_edges(cfg, ei)

    key = (cfg.N, cfg.E, MODE, tuple(plan["G"].tolist()))
    if key not in _CACHE:
        _CACHE.clear()
        _CACHE[key] = build_kernel(cfg, plan["G"], plan["g_off"], plan["NG"],
                                   plan["bin_rank"])
    nc = _CACHE[key]

    # node-major fp16 gather table (padded, quarter-major chunk layout)
    xg = np.zeros((cfg.NTOT, 128), dtype=np.float16)
    for k in range(cfg.NC):
        for q in range(cfg.NCHUNK):
            nreal = min(cfg.qrows[q], max(0, cfg.PCR - cfg.qsrow[q]))
            if nreal <= 0:
                continue
            d0 = cfg.cstart[q] + k * cfg.qrows[q]
            s0 = k * cfg.PCR + cfg.qsrow[q]
            xg[d0:d0 + nreal] = x[s0:s0 + nreal].astype(np.float16)

    def w16(a):
        return np.ascontiguousarray(np.asarray(a, np.float32).T).astype(np.float16)

    clfT = np.zeros((128, 128), dtype=np.float16)
    clfT[:, :64] = w16(inputs["clf_w"])
    par = np.zeros((128, 10), dtype=np.float32)
    par[:, 0] = np.asarray(inputs["s3_b"], np.float32)
    par[:, 1] = np.asarray(inputs["ff1_b"], np.float32)
    par[:, 2] = np.asarray(inputs["ff2_b"], np.float32)
    par[:64, 3] = np.asarray(inputs["clf_b"], np.float32)
    par[:, 4] = np.asarray(inputs["bn1_g"], np.float32)
    par[:, 5] = np.asarray(inputs["bn1_b"], np.float32)
    par[:, 6] = np.asarray(inputs["bn2_g"], np.float32)
    par[:, 7] = np.asarray(inputs["bn2_b"], np.float32)
    par[:, 8] = np.asarray(inputs["res1_b"], np.float32)
    par[:, 9] = np.asarray(inputs["res2_b"], np.float32)

    weights = dict(
        wl1T=w16(inputs["s1_wl"]), wr1T=w16(inputs["s1_wr"]),
        res1T=w16(inputs["res1_w"]),
        wl2T=w16(inputs["s2_wl"]), wr2T=w16(inputs["s2_wr"]),
        res2T=w16(inputs["res2_w"]),
        wl3T=w16(inputs["s3_wl"]), wr3T=w16(inputs["s3_wr"]),
        ff1T=w16(inputs["ff1_w"]), ff2T=w16(inputs["ff2_w"]), clfT=clfT)

    in_maps = []
    for k in range(cfg.NC):
        xslice = np.zeros((128, cfg.NPC), dtype=np.float16)
        xslice[:, :cfg.PCR] = x[k * cfg.PCR:(k + 1) * cfg.PCR].T.astype(np.float16)
        im = dict(xg=xg, xT=xslice, idx=plan["idx"][k], mmoh=plan["mmoh"][k],
                  invdeg=plan["inv"][k], nreal=plan["nreal"][k], par=par,
                  **weights)
        in_maps.append(im)
    return cfg, nc, in_maps


_LAST_EXEC_NS = None
_LAST_RESULTS = None


def _ensure_axon_hooks():
    """The image's antenv lacks axon_hooks; shim it so NTFF tracing works."""
    import sys
    import types
    try:
        import antenv.axon_hooks  # noqa: F401
        return
    except ImportError:
        pass
    import antenv
    mod = types.ModuleType("antenv.axon_hooks")
    mod._hook = None

    def set_axon_ntff_profile_hook(h):
        mod._hook = h

    def get_axon_ntff_profile_hook():
        return mod._hook

    mod.set_axon_ntff_profile_hook = set_axon_ntff_profile_hook
    mod.get_axon_ntff_profile_hook = get_axon_ntff_profile_hook
    sys.modules["antenv.axon_hooks"] = mod
    antenv.axon_hooks = mod
    try:
        from trn_agent_boot.trn_boot import _ntff_profile_via_ctypes
        so = "/opt/axon/libaxon_pjrt.so"
        if os.path.exists(so):
            mod._hook = _ntff_profile_via_ctypes(so)
    except Exception as e:  # pragma: no cover
        print("ntff hook shim failed:", e)


def kernel(**inputs) -> np.ndarray:
    global _LAST_EXEC_NS, _LAST_RESULTS
    cfg, nc, in_maps = _prepare(inputs)
    trace = bool(int(os.environ.get("BASS_GNN_TRACE", "0")))
    if trace:
        _ensure_axon_hooks()
    res = bass_utils.run_bass_kernel_spmd(
        nc, in_maps, core_ids=list(range(cfg.NC)), trace=trace)
    _LAST_EXEC_NS = res.exec_time_ns
    _LAST_RESULTS = res
    out = np.empty((cfg.N, 64), dtype=np.float32)
    for k in range(cfg.NC):
        out[k * cfg.PCR:(k + 1) * cfg.PCR] = res.results[k]["out"][:cfg.PCR]
    return out



# revision 50
# speedup vs baseline: 1.0565x; 1.0268x over previous
"""Trainium2 Bass kernel for EnhancedSAGEModel (3x SAGEConv mean-aggr + BN + FFN head).

Sharding: nodes across 8 cores (12500 real + 44 pad slots each); edges
partitioned by destination; per-layer dma_gather of source rows from a
replicated node-feature table (fp16, quarter-major chunk layout), scatter-add
via one-hot matmul into PSUM in feature-major layout; chunked AllGather of
activations between layers (quarter q issues as soon as pass_b wrote it);
AllReduce for BatchNorm batch statistics.

Perf notes (6.56ms -> ~2.7ms on 8 axon trn2 cores):
- gathers spread over all 4 SWDGE queues (num_swdge_queues=4, queue_num=c%4):
  each queue runs on its own Q7 core pair; descriptor generation is the
  kernel's pacing resource (~8ns/idx per pair).
- the one-hot scatter matrix is precomputed on the host in fp8e4 (0/1 exact)
  and DMA-loaded via HWDGE; matmuls run mixed fp16(lhsT=xe) x fp8(rhs).
- per-(block,chunk) trailing pad indices are -1 with the per-core real count
  passed via num_idxs_reg (rotating registers): the gather ucode skips them.
- xe pool buffers are zero-initialized once (trimmed slots are still read by
  matmuls against 0.0 one-hot columns; stale NaN * 0 would poison PSUM).
- layer-3 FFN+classifier head is fused per block into the scatter pass.

Self-contained: hardcodes the problem shapes from spec
(x:[100000,128] f32, edge_index:[2,1600000] i64, weights 128x128 etc.).
"""

import math
import os
import numpy as np
import ml_dtypes

import concourse.bass as bass
import concourse.bacc as bacc
import concourse.mybir as mybir
import concourse.tile as tile
from concourse.bass import ts
from concourse import bass_utils

F16 = mybir.dt.float16
F32 = mybir.dt.float32
F8 = mybir.dt.float8e4
NP_F8 = ml_dtypes.float8_e4m3
I16 = mybir.dt.int16

# mm8: one-hot in fp8 (exact), xe fp16, plain matmuls (default)
# dr8: one-hot + xe in fp8, DoubleRow paired matmuls (fast, ~1.4e-2 rel err)
# fp16: everything fp16
MODE = os.environ.get("BASS_GNN_MODE", "mm8")
assert MODE in ("mm8", "dr8", "fp16")
USE_FP8 = MODE == "dr8"          # xe cast + DoubleRow
MM_FP8 = MODE in ("mm8", "dr8")  # one-hot dtype

AX = mybir.AxisListType
ALU = mybir.AluOpType
ACTF = mybir.ActivationFunctionType

EPS = 1e-5


class Cfg:
    """Quarter-major table layout: global table row of node (core k, local i)
    with i in quarter q (by window) is cstart[q] + k*qrows[q] + (i - qsrow[q]).
    Chunk c == quarter c, so the next-layer gather of chunk c only depends on
    the AllGather of quarter c (enables chunked AG overlap with pass_b)."""

    def __init__(self, n_real, e, n_cores=8, d=128, c_out=64):
        self.N = n_real
        self.E = e
        self.NC = n_cores
        self.D = d
        self.C = c_out
        assert n_real % n_cores == 0
        self.PCR = n_real // n_cores              # real nodes per core
        self.NW = math.ceil(self.PCR / 128)       # windows per core (128 dst each)
        self.NPC = self.NW * 128                  # padded nodes per core
        self.NTOT = self.NPC * n_cores            # padded global nodes
        self.NCHUNK = 4
        base, rem = divmod(self.NW, self.NCHUNK)
        self.QW = [base + (1 if i < rem else 0) for i in range(self.NCHUNK)]
        assert self.QW[0] * 128 * n_cores <= 32767
        self.QWS = np.concatenate([[0], np.cumsum(self.QW)])  # window starts
        self.qrows = [w * 128 for w in self.QW]               # rows per quarter
        self.qsrow = [int(s) * 128 for s in self.QWS[:-1]]    # local row starts
        self.csize = [n_cores * r for r in self.qrows]        # chunk sizes
        self.cstart = [0]
        for s in self.csize[:-1]:
            self.cstart.append(self.cstart[-1] + s)
        for cs in self.csize:
            assert cs <= 32767, "int16 gather index limit"
        self.NB = math.ceil(self.NW / 4)          # dense blocks (<=4 windows each)
        # pass_b block after which quarter q's h_loc rows are all written
        self.q_done_block = [math.ceil(int(self.QWS[q + 1]) / 4) - 1
                             for q in range(self.NCHUNK)]

    def block_windows(self, b):
        return range(4 * b, min(4 * b + 4, self.NW))


def plan_edges(cfg, edge_index):
    """Bin edges per core by (dense-block, chunk, window); pad each bin to a
    multiple of 128 slots. Returns per-core plan arrays + shared structure."""
    src = np.asarray(edge_index[0], dtype=np.int64)
    dst = np.asarray(edge_index[1], dtype=np.int64)
    NC, PCR, NPC = cfg.NC, cfg.PCR, cfg.NPC
    NW, NCH = cfg.NW, cfg.NCHUNK

    deg = np.bincount(dst, minlength=cfg.N).astype(np.float32)
    invdeg_full = (1.0 / np.maximum(deg, 1.0)).astype(np.float32)

    core = dst // PCR
    ld_all = dst % PCR                       # local dst in [0, PCR)
    w_all = ld_all // 128
    ldw_all = (ld_all % 128).astype(np.float16)
    # quarter-major chunk mapping of sources
    k_src = src // PCR
    i_src = src % PCR
    c_all = np.searchsorted(cfg.QWS[1:], i_src // 128, side="right")
    qrows_a = np.asarray(cfg.qrows, dtype=np.int64)
    qsrow_a = np.asarray(cfg.qsrow, dtype=np.int64)
    lsrc_all = (k_src * qrows_a[c_all] + (i_src - qsrow_a[c_all])).astype(
        np.int16)

    # bin linear order: for block: for chunk: for window-in-block
    # bin_rank(w, c) -> position in that order
    bin_rank = np.empty((NW, NCH), dtype=np.int64)
    r = 0
    order = []
    for b in range(cfg.NB):
        for c in range(NCH):
            for w in cfg.block_windows(b):
                bin_rank[w, c] = r
                order.append((w, c))
                r += 1
    NBINS = r

    counts = np.zeros((NC, NBINS), dtype=np.int64)
    binid = bin_rank[w_all, c_all]
    for k in range(NC):
        m = core == k
        counts[k] = np.bincount(binid[m], minlength=NBINS)

    G = np.maximum(np.ceil(counts.max(axis=0) / 128.0).astype(np.int64), 0)
    g_off = np.concatenate([[0], np.cumsum(G)])  # group offset per bin
    NG = int(g_off[-1])

    mm_np = NP_F8 if MM_FP8 else np.float16
    idx_plans, mm_plans, inv_plans, nreal_plans = [], [], [], []
    for k in range(NC):
        m = core == k
        bk = binid[m]
        lsrc_k = lsrc_all[m]
        ldw_k = ldw_all[m]
        ordk = np.argsort(bk, kind="stable")
        bk = bk[ordk]
        lsrc_k = lsrc_k[ordk]
        ldw_k = ldw_k[ordk].astype(np.int64)
        starts = np.concatenate([[0], np.cumsum(counts[k])])
        rank = np.arange(len(bk)) - starts[bk]

        idx_arr = np.zeros((128, NG * 8), dtype=np.int16)
        # Trailing pad of the LAST nonempty bin of each (block, chunk) gather
        # call: mark idx=-1 and record the per-core real count (passed as
        # num_idxs_reg) so the ucode skips those descriptors at runtime.
        nreal = np.zeros(cfg.NB * NCH, dtype=np.int32)
        for b in range(cfg.NB):
            wins_b = list(cfg.block_windows(b))
            for c in range(NCH):
                gb0 = g_off[bin_rank[wins_b[0], c]]
                ng_call = int(sum(G[bin_rank[w, c]] for w in wins_b))
                if ng_call == 0:
                    continue
                wl = max(w for w in wins_b if G[bin_rank[w, c]] > 0)
                bin_ = bin_rank[wl, c]
                gb, ge = g_off[bin_], g_off[bin_ + 1]
                pad_r = np.arange(counts[k][bin_], (ge - gb) * 128)
                idx_arr[pad_r % 16, gb * 8 + pad_r // 16] = -1
                nreal[b * NCH + c] = (gb - gb0) * 128 + counts[k][bin_]
        col = g_off[bk] * 8 + rank // 16
        idx_arr[rank % 16, col] = lsrc_k
        # HW ucode: each of the 8 Q7 cores reads its own 16-partition group
        idx_arr = np.tile(idx_arr[:16], (8, 1))
        # host-precomputed one-hot scatter matrix: slot (g*128+p) -> dst d
        slot = g_off[bk] * 128 + rank
        mm_arr = np.zeros((128, NG * 128), dtype=mm_np)
        mm_arr[slot % 128, (slot // 128) * 128 + ldw_k] = 1.0

        inv = np.ones((NPC,), dtype=np.float32)
        inv[:PCR] = invdeg_full[k * PCR:(k + 1) * PCR]
        inv_plans.append(inv.reshape(NW, 128))
        idx_plans.append(idx_arr)
        mm_plans.append(mm_arr)
        nreal_plans.append(nreal.reshape(1, -1))

    return dict(G=G, g_off=g_off, NG=NG, bin_rank=bin_rank,
                idx=idx_plans, mmoh=mm_plans, inv=inv_plans, nreal=nreal_plans)


def build_kernel(cfg, G, g_off, NG, bin_rank):
    """Build the 8-core SPMD Bass program. Structure (G table) is compile-time."""
    NW, NCH, NPC, NTOT, NB = cfg.NW, cfg.NCHUNK, cfg.NPC, cfg.NTOT, cfg.NB
    N_real = cfg.N

    nc = bacc.Bacc("TRN2", target_bir_lowering=False, debug=False,
                   num_devices=cfg.NC, num_swdge_queues=4)
    rg = [list(range(cfg.NC))]

    # ---- I/O ----
    F8M = F8 if MM_FP8 else F16
    xg = nc.dram_tensor("xg", [NTOT, 128], F16, kind="ExternalInput")
    xT = nc.dram_tensor("xT", [128, NPC], F16, kind="ExternalInput")
    idx_d = nc.dram_tensor("idx", [128, NG * 8], I16, kind="ExternalInput")
    mm_d = nc.dram_tensor("mmoh", [128, NG * 128], F8M, kind="ExternalInput")
    inv_d = nc.dram_tensor("invdeg", [NW, 128], F32, kind="ExternalInput")
    nreal_d = nc.dram_tensor("nreal", [1, NB * NCH], mybir.dt.int32,
                             kind="ExternalInput")
    wnames = ["wl1T", "wr1T", "res1T", "wl2T", "wr2T", "res2T",
              "wl3T", "wr3T", "ff1T", "ff2T", "clfT"]
    wd = {n: nc.dram_tensor(n, [128, 128], F16, kind="ExternalInput")
          for n in wnames}
    # packed per-partition params: col 0:b3,1:ff1b,2:ff2b,3:clfb,
    # 4:bn1g,5:bn1b,6:bn2g,7:bn2b,8:res1b,9:res2b
    par_d = nc.dram_tensor("par", [128, 10], F32, kind="ExternalInput")
    out_d = nc.dram_tensor("out", [NPC, 64], F32, kind="ExternalOutput")

    h_loc = [nc.dram_tensor(f"h_loc{i}", [NPC, 128], F16, kind="Internal")
             for i in range(2)]
    hTd = [nc.dram_tensor(f"hTd{i}", [128, NPC], F16, kind="Internal")
           for i in range(2)]
    h_full = [nc.dram_tensor(f"h_full{i}", [NTOT, 128], F16, kind="Internal",
                             addr_space="Shared") for i in range(2)]
    st_in = [nc.dram_tensor(f"st_in{i}", [128, 2], F32, kind="Internal")
             for i in range(2)]
    st_out = [nc.dram_tensor(f"st_out{i}", [128, 2], F32, kind="Internal",
                             addr_space="Shared") for i in range(2)]

    xe_groups_max = max(
        int(sum(G[bin_rank[w, c]] for w in cfg.block_windows(b)))
        for b in range(NB) for c in range(NCH)) or 1

    with tile.TileContext(nc) as tc:
        with (
            tc.tile_pool(name="persist", bufs=1) as persist,
            tc.tile_pool(name="xe_p", bufs=10) as xe_p,
            tc.tile_pool(name="x8_p", bufs=8) as x8_p,
            tc.tile_pool(name="m_p", bufs=10) as m_p,
            tc.tile_pool(name="sm", bufs=3) as sm,
            tc.tile_pool(name="smb", bufs=2) as smb,
            tc.tile_pool(name="hp_p", bufs=2) as hp_p,
            tc.tile_pool(name="agg_pp", bufs=2, space="PSUM") as agg_pp,
            tc.tile_pool(name="z_pp", bufs=2, space="PSUM") as z_pp,
            tc.tile_pool(name="r_pp", bufs=2, space="PSUM") as r_pp,
            tc.tile_pool(name="t_pp", bufs=2, space="PSUM") as t_pp,
        ):
            # ---- persistent loads ----
            idx_sb = persist.tile([128, NG * 8], I16)
            nc.sync.dma_start(out=idx_sb[:, :], in_=idx_d[:, :])
            nreal_sb = persist.tile([1, NB * NCH], mybir.dt.int32)
            nc.sync.dma_start(out=nreal_sb[:, :], in_=nreal_d[:, :])
            nreal_regs = [nc.gpsimd.alloc_register(f"nreal_r{i}")
                          for i in range(8)]
            w_sb = {}
            for n in wnames:
                w_sb[n] = persist.tile([128, 128], F16, name=f"w_{n}")
                nc.sync.dma_start(out=w_sb[n][:, :], in_=wd[n][:, :])
            par_sb = persist.tile([128, 10], F32)
            nc.sync.dma_start(out=par_sb[:, :], in_=par_d[:, :])
            eps_sb = persist.tile([128, 1], F32)
            nc.vector.memset(eps_sb[:, :], EPS)
            # bn affine params per layer: cols 0=scale,1=bias,2=mean,3=tmp,4=tmp2
            bnp_sb = persist.tile([128, 6], F32)

            # zero-init the xe pool buffers: slots trimmed by trailing -1
            # indices are still read by matmuls (times a 0.0 one-hot column),
            # and uninitialized SBUF bits could decode as NaN/Inf (0*NaN=NaN).
            for _ in range(10):
                t0_ = xe_p.tile([128, xe_groups_max, 128], F16, name="xe",
                                tag="xe",
                                padded_shape=[128, xe_groups_max, 128])
                nc.vector.memset(t0_[:, :, :], 0.0)

            z_sb = persist.tile([128, NPC], F16)
            stats_sb = persist.tile([128, 2], F32)
            stats_in_sb = persist.tile([128, 2], F32)
            sums_sb = persist.tile([128, NB], F32)
            sqs_sb = persist.tile([128, NB], F32)

            def scatter_pass(layer, table_ap, hprev_d):
                """Pass A: aggregate + dense matmuls -> z_sb (+ stats).

                hprev_d: DRAM [128, NPC] fp16 (feature-major prev acts)."""
                for b in range(NB):
                    wins = list(cfg.block_windows(b))
                    nwin = len(wins)
                    agg = agg_pp.tile([128, nwin * 128], F32, name="agg",
                                      tag="agg", padded_shape=[128, 512])
                    # gather + cast + one-hot load for all chunks of this block
                    chunk_tiles = [None] * NCH
                    for c in range(NCH):
                        gb0 = int(g_off[bin_rank[wins[0], c]])
                        ng = int(sum(G[bin_rank[w, c]] for w in wins))
                        if ng == 0:
                            continue
                        nidx = ng * 128
                        xe = xe_p.tile([128, ng, 128], F16, name="xe", tag="xe",
                                       padded_shape=[128, xe_groups_max, 128])
                        ci = b * NCH + c
                        cnt = nreal_regs[ci % 8]
                        nc.gpsimd.reg_load(cnt, nreal_sb[0:1, ci:ci + 1])
                        nc.gpsimd.dma_gather(
                            out_ap=xe[:, :, :],
                            in_ap=table_ap[cfg.cstart[c]:
                                           cfg.cstart[c] + cfg.csize[c], :],
                            idxs_ap=idx_sb[:, gb0 * 8:(gb0 + ng) * 8],
                            num_idxs=nidx, num_idxs_reg=cnt, elem_size=128,
                            single_packet=False, queue_num=c % 4)
                        mm = m_p.tile([128, ng, 128], F8M, name="mm", tag="mm",
                                      padded_shape=[128, xe_groups_max, 128])
                        nc.sync.dma_start(
                            out=mm[:, :, :],
                            in_=mm_d[:, gb0 * 128:(gb0 + ng) * 128].rearrange(
                                "p (g f) -> p g f", g=ng))
                        if USE_FP8:
                            x8 = x8_p.tile([128, ng, 128], F8, name="x8",
                                           tag="x8",
                                           padded_shape=[128, xe_groups_max,
                                                         128])
                            nc.vector.tensor_copy(out=x8[:, :, :],
                                                  in_=xe[:, :, :])
                        else:
                            x8 = xe
                        chunk_tiles[c] = (x8, mm, gb0)
                    # matmuls: each window's accumulation contiguous so PSUM
                    # zero-region (full bank) never has two open groups
                    for wi, w in enumerate(wins):
                        entries = []  # (x8, mm, gi0, G)
                        for c in range(NCH):
                            if chunk_tiles[c] is None:
                                continue
                            gwc = int(G[bin_rank[w, c]])
                            if gwc == 0:
                                continue
                            x8, mm, gb0 = chunk_tiles[c]
                            gi0 = int(g_off[bin_rank[w, c]]) - gb0
                            entries.append((x8, mm, gi0, gwc))
                        if not entries:
                            continue
                        # (tile, g, span): span 2 => fp8 DoubleRow pair
                        mms = []
                        for x8, mm, gi0, gwc in entries:
                            g = 0
                            while g < gwc:
                                span = 2 if (USE_FP8 and g + 1 < gwc) else 1
                                mms.append((x8, mm, gi0 + g, span))
                                g += span
                        for j, (x8, mm, g0, span) in enumerate(mms):
                            if span == 2:
                                nc.tensor.matmul(
                                    agg[:, ts(wi, 128)],
                                    lhsT=x8[:, g0:g0 + 2, :],
                                    rhs=mm[:, g0:g0 + 2, :],
                                    start=(j == 0), stop=(j == len(mms) - 1),
                                    perf_mode=mybir.MatmulPerfMode.DoubleRow)
                            else:
                                nc.tensor.matmul(
                                    agg[:, ts(wi, 128)],
                                    lhsT=x8[:, g0, :],
                                    rhs=mm[:, g0, :],
                                    start=(j == 0), stop=(j == len(mms) - 1))
                    # evict: mean_T = agg * invdeg (broadcast over partitions)
                    invB = smb.tile([128, nwin * 128], F32, name="invB",
                                    tag="invB", padded_shape=[128, 512])
                    inv_ap = bass.AP(tensor=inv_d, offset=wins[0] * 128,
                                     ap=[[0, 128], [1, nwin * 128]])
                    nc.sync.dma_start(out=invB[:, :], in_=inv_ap)
                    mean = sm.tile([128, nwin * 128], F16, name="mean",
                                   tag="mean", padded_shape=[128, 512])
                    empty = [wi for wi, w in enumerate(wins)
                             if all(G[bin_rank[w, c]] == 0 for c in range(NCH))]
                    if len(empty) == nwin:
                        nc.vector.memset(mean[:, :], 0.0)
                    else:
                        nc.vector.tensor_tensor(out=mean[:, :], in0=agg[:, :],
                                                in1=invB[:, :], op=ALU.mult)
                        for wi in empty:
                            nc.vector.memset(mean[:, ts(wi, 128)], 0.0)
                    # dense: z = WlT.T@mean + WrT.T@hprev
                    wl, wr = (("wl1T", "wr1T"), ("wl2T", "wr2T"),
                              ("wl3T", "wr3T"))[layer]
                    hp = hp_p.tile([128, nwin * 128], F16, name="hp",
                                   tag="hp", padded_shape=[128, 512])
                    nc.sync.dma_start(
                        out=hp[:, :],
                        in_=hprev_d[:, b * 512:b * 512 + nwin * 128])
                    zp = z_pp.tile([128, nwin * 128], F32, name="zp", tag="zp",
                                   padded_shape=[128, 512])
                    nc.tensor.matmul(zp[:, :], lhsT=w_sb[wl][:, :],
                                     rhs=mean[:, :], start=True, stop=False)
                    nc.tensor.matmul(zp[:, :], lhsT=w_sb[wr][:, :],
                                     rhs=hp[:, :], start=False, stop=True)
                    if layer < 2:
                        # evict to fp16 z, accumulate sum + sumsq partials
                        nc.scalar.activation(z_sb[:, b * 512:b * 512 + nwin * 128],
                                             zp[:, :], ACTF.Copy,
                                             accum_out=sums_sb[:, b:b + 1])
                        sq = sm.tile([128, nwin * 128], F16, name="sq",
                                     tag="sq", padded_shape=[128, 512])
                        nc.scalar.activation(sq[:, :], zp[:, :], ACTF.Square,
                                             accum_out=sqs_sb[:, b:b + 1])
                    else:
                        # layer 3: z + b3 directly, no BN; head fused per block
                        nc.scalar.activation(z_sb[:, b * 512:b * 512 + nwin * 128],
                                             zp[:, :], ACTF.Identity,
                                             bias=par_sb[:, 0:1], scale=1.0)
                        head_block(b)

            def bn_params(layer):
                """AllReduce stats; compute scale/bias cols in bnp_sb."""
                si, so = st_in[layer], st_out[layer]
                nc.vector.reduce_sum(stats_in_sb[:, 0:1], sums_sb[:, :],
                                     axis=AX.X)
                nc.vector.reduce_sum(stats_in_sb[:, 1:2], sqs_sb[:, :],
                                     axis=AX.X)
                nc.sync.dma_start(out=si[:, :], in_=stats_in_sb[:, :])
                nc.gpsimd.collective_compute(
                    "AllReduce", ALU.add, replica_groups=rg,
                    ins=[si[:, :]], outs=[so[:, :]])
                nc.sync.dma_start(out=stats_sb[:, :], in_=so[:, :])
                g_ap = par_sb[:, 4 + 2 * layer:5 + 2 * layer]
                beta_ap = par_sb[:, 5 + 2 * layer:6 + 2 * layer]
                mean_ap = bnp_sb[:, 2:3]
                tmp_ap = bnp_sb[:, 3:4]
                tmp2_ap = bnp_sb[:, 4:5]
                # mean = s0/N ; ez2 = s1/N
                nc.scalar.activation(mean_ap, stats_sb[:, 0:1], ACTF.Copy,
                                     scale=1.0 / N_real)
                nc.scalar.activation(tmp_ap, stats_sb[:, 1:2], ACTF.Copy,
                                     scale=1.0 / N_real)
                # var = ez2 - mean^2
                nc.vector.tensor_tensor(out=tmp2_ap, in0=mean_ap, in1=mean_ap,
                                        op=ALU.mult)
                nc.vector.tensor_tensor(out=tmp_ap, in0=tmp_ap, in1=tmp2_ap,
                                        op=ALU.subtract)
                # rstd = 1/sqrt(var + eps)
                nc.scalar.activation(tmp_ap, tmp_ap, ACTF.Sqrt,
                                     bias=eps_sb[:, 0:1])
                nc.vector.reciprocal(tmp_ap, tmp_ap)
                # scale = rstd*g ; bias = beta - mean*scale
                nc.vector.tensor_tensor(out=bnp_sb[:, 0:1], in0=tmp_ap,
                                        in1=g_ap, op=ALU.mult)
                nc.vector.tensor_tensor(out=tmp2_ap, in0=mean_ap,
                                        in1=bnp_sb[:, 0:1], op=ALU.mult)
                nc.vector.tensor_tensor(out=bnp_sb[:, 1:2], in0=beta_ap,
                                        in1=tmp2_ap, op=ALU.subtract)

            def pass_b(layer, hprev_d, hnew_d, hloc, ag_out):
                """relu(bn(z)) + res -> hnew (fp16, DRAM); transpose+write h_loc.

                Issues the quarter-q AllGather (hloc rows -> ag_out chunk q)
                as soon as the blocks covering quarter q are written."""
                resw = ("res1T", "res2T")[layer]
                for b in range(NB):
                    wins = list(cfg.block_windows(b))
                    nwin = len(wins)
                    bs = b * 512
                    hp = hp_p.tile([128, nwin * 128], F16, name="hpb",
                                   tag="hp", padded_shape=[128, 512])
                    nc.sync.dma_start(out=hp[:, :],
                                      in_=hprev_d[:, bs:bs + nwin * 128])
                    rp = r_pp.tile([128, nwin * 128], F32, name="rp", tag="rp",
                                   padded_shape=[128, 512])
                    nc.tensor.matmul(rp[:, :], lhsT=w_sb[resw][:, :],
                                     rhs=hp[:, :], start=True, stop=True)
                    hbuf = sm.tile([128, nwin * 128], F32, name="hbuf",
                                   tag="hbuf", padded_shape=[128, 512])
                    nc.scalar.activation(hbuf[:, :], z_sb[:, bs:bs + nwin * 128],
                                         ACTF.Relu, bias=bnp_sb[:, 1:2],
                                         scale=bnp_sb[:, 0:1])
                    hf = sm.tile([128, nwin * 128], F32, name="hf", tag="hf",
                                 padded_shape=[128, 512])
                    # hf = (hbuf + res_bias) + res_matmul
                    nc.vector.scalar_tensor_tensor(
                        out=hf[:, :], in0=hbuf[:, :],
                        scalar=par_sb[:, 8 + layer:9 + layer],
                        in1=rp[:, :], op0=ALU.add, op1=ALU.add)
                    if b == NB - 1 and NPC > cfg.PCR:
                        # zero pad-node columns (keeps next-layer stats clean)
                        pstart = cfg.PCR - bs
                        nc.vector.memset(hf[:, pstart:nwin * 128], 0.0)
                    h16 = sm.tile([128, nwin * 128], F16, name="h16",
                                  tag="h16", padded_shape=[128, 512])
                    nc.vector.tensor_copy(out=h16[:, :], in_=hf[:, :])
                    nc.sync.dma_start(out=hnew_d[:, bs:bs + nwin * 128],
                                      in_=h16[:, :])
                    # transpose to node-major and store
                    tp = t_pp.tile([128, nwin * 128], F32, name="tp", tag="tp",
                                   padded_shape=[128, 512])
                    for wi in range(nwin):
                        nc.tensor.transpose(tp[:, ts(wi, 128)],
                                            hf[:, ts(wi, 128)],
                                            iden_sb[:, :])
                    wb = sm.tile([128, nwin * 128], F16, name="wb", tag="wb",
                                 padded_shape=[128, 512])
                    nc.vector.tensor_copy(out=wb[:, :], in_=tp[:, :])
                    dst_ap = bass.AP(
                        tensor=hloc, offset=bs * 128,
                        ap=[[128, 128], [128 * 128, nwin], [1, 128]])
                    nc.sync.dma_start(out=dst_ap, in_=wb[:, :].rearrange(
                        "p (w f) -> p w f", w=nwin))
                    for q in range(NCH):
                        if cfg.q_done_block[q] == b:
                            nc.gpsimd.collective_compute(
                                "AllGather", ALU.bypass, replica_groups=rg,
                                ins=[hloc[cfg.qsrow[q]:
                                          cfg.qsrow[q] + cfg.qrows[q], :]],
                                outs=[ag_out[cfg.cstart[q]:
                                             cfg.cstart[q] + cfg.csize[q], :]])

            def head_block(b):
                """relu(ff1@z+b) -> ff2 -> clf; transpose; write out (one block)."""
                if True:
                    wins = list(cfg.block_windows(b))
                    nwin = len(wins)
                    bs = b * 512
                    q1p = z_pp.tile([128, nwin * 128], F32, name="q1p",
                                    tag="zp", padded_shape=[128, 512])
                    nc.tensor.matmul(q1p[:, :], lhsT=w_sb["ff1T"][:, :],
                                     rhs=z_sb[:, bs:bs + nwin * 128],
                                     start=True, stop=True)
                    q1 = sm.tile([128, nwin * 128], F16, name="q1", tag="mean",
                                 padded_shape=[128, 512])
                    nc.scalar.activation(q1[:, :], q1p[:, :], ACTF.Relu,
                                         bias=par_sb[:, 1:2])
                    q2p = r_pp.tile([128, nwin * 128], F32, name="q2p",
                                    tag="rp", padded_shape=[128, 512])
                    nc.tensor.matmul(q2p[:, :], lhsT=w_sb["ff2T"][:, :],
                                     rhs=q1[:, :], start=True, stop=True)
                    q2 = sm.tile([128, nwin * 128], F16, name="q2", tag="sq",
                                 padded_shape=[128, 512])
                    nc.scalar.activation(q2[:, :], q2p[:, :], ACTF.Identity,
                                         bias=par_sb[:, 2:3])
                    op_ = z_pp.tile([128, nwin * 128], F32, name="op_",
                                    tag="zp", padded_shape=[128, 512])
                    nc.tensor.matmul(op_[:, :], lhsT=w_sb["clfT"][:, :],
                                     rhs=q2[:, :], start=True, stop=True)
                    ob = sm.tile([128, nwin * 128], F32, name="ob", tag="hbuf",
                                 padded_shape=[128, 512])
                    nc.scalar.activation(ob[:, :], op_[:, :], ACTF.Identity,
                                         bias=par_sb[:, 3:4])
                    tp = t_pp.tile([128, nwin * 128], F32, name="tp2", tag="tp",
                                   padded_shape=[128, 512])
                    for wi in range(nwin):
                        nc.tensor.transpose(tp[:, ts(wi, 128)],
                                            ob[:, ts(wi, 128)],
                                            iden_sb[:, :])
                    owb = sm.tile([128, nwin * 64], F32, name="owb", tag="hf",
                                  padded_shape=[128, 512])
                    for wi in range(nwin):
                        nc.vector.tensor_copy(out=owb[:, ts(wi, 64)],
                                              in_=tp[:, wi * 128:wi * 128 + 64])
                    dst_ap = bass.AP(
                        tensor=out_d, offset=bs * 64,
                        ap=[[64, 128], [128 * 64, nwin], [1, 64]])
                    nc.sync.dma_start(out=dst_ap, in_=owb[:, :].rearrange(
                        "p (w f) -> p w f", w=nwin))

            from concourse.masks import make_identity
            iden_sb = persist.tile([128, 128], F32)
            make_identity(nc, iden_sb[:, :])

            # ================= layer 1 =================
            scatter_pass(0, xg[:, :], xT[:, :])
            bn_params(0)
            pass_b(0, xT[:, :], hTd[0][:, :], h_loc[0], h_full[0])
            # ================= layer 2 =================
            scatter_pass(1, h_full[0][:, :], hTd[0][:, :])
            bn_params(1)
            pass_b(1, hTd[0][:, :], hTd[1][:, :], h_loc[1], h_full[1])
            # ================= layer 3 + head (fused) ==
            scatter_pass(2, h_full[1][:, :], hTd[1][:, :])

            if os.environ.get("BASS_GNN_DEBUG"):
                dbg = nc.dram_tensor("dbg_h1", [NTOT, 128], F16,
                                     kind="ExternalOutput")
                with tc.tile_pool(name="dbgp", bufs=2) as dbgp:
                    for i in range(NTOT // 128):
                        dt_ = dbgp.tile([128, 128], F16, name="dt_", tag="dt")
                        nc.sync.dma_start(
                            out=dt_[:, :],
                            in_=h_full[0][i * 128:(i + 1) * 128, :])
                        nc.sync.dma_start(
                            out=dbg[i * 128:(i + 1) * 128, :], in_=dt_[:, :])

    nc.compile()
    return nc


# ---------------- host side ----------------

_CACHE = {}


def _prepare(inputs):
    x = np.asarray(inputs["x"], dtype=np.float32)
    ei = np.asarray(inputs["edge_index"])
    cfg = Cfg(n_real=x.shape[0], e=ei.shape[1])
    plan = plan_edges(cfg, ei)

    key = (cfg.N, cfg.E, MODE, tuple(plan["G"].tolist()))
    if key not in _CACHE:
        _CACHE.clear()
        _CACHE[key] = build_kernel(cfg, plan["G"], plan["g_off"], plan["NG"],
                                   plan["bin_rank"])
    nc = _CACHE[key]

    # node-major fp16 gather table (padded, quarter-major chunk layout)
    xg = np.zeros((cfg.NTOT, 128), dtype=np.float16)
    for k in range(cfg.NC):
        for q in range(cfg.NCHUNK):
            nreal = min(cfg.qrows[q], max(0, cfg.PCR - cfg.qsrow[q]))
            if nreal <= 0:
                continue
            d0 = cfg.cstart[q] + k * cfg.qrows[q]
            s0 = k * cfg.PCR + cfg.qsrow[q]
            xg[d0:d0 + nreal] = x[s0:s0 + nreal].astype(np.float16)

    def w16(a):
        return np.ascontiguousarray(np.asarray(a, np.float32).T).astype(np.float16)

    clfT = np.zeros((128, 128), dtype=np.float16)
    clfT[:, :64] = w16(inputs["clf_w"])
    par = np.zeros((128, 10), dtype=np.float32)
    par[:, 0] = np.asarray(inputs["s3_b"], np.float32)
    par[:, 1] = np.asarray(inputs["ff1_b"], np.float32)
    par[:, 2] = np.asarray(inputs["ff2_b"], np.float32)
    par[:64, 3] = np.asarray(inputs["clf_b"], np.float32)
    par[:, 4] = np.asarray(inputs["bn1_g"], np.float32)
    par[:, 5] = np.asarray(inputs["bn1_b"], np.float32)
    par[:, 6] = np.asarray(inputs["bn2_g"], np.float32)
    par[:, 7] = np.asarray(inputs["bn2_b"], np.float32)
    par[:, 8] = np.asarray(inputs["res1_b"], np.float32)
    par[:, 9] = np.asarray(inputs["res2_b"], np.float32)

    weights = dict(
        wl1T=w16(inputs["s1_wl"]), wr1T=w16(inputs["s1_wr"]),
        res1T=w16(inputs["res1_w"]),
        wl2T=w16(inputs["s2_wl"]), wr2T=w16(inputs["s2_wr"]),
        res2T=w16(inputs["res2_w"]),
        wl3T=w16(inputs["s3_wl"]), wr3T=w16(inputs["s3_wr"]),
        ff1T=w16(inputs["ff1_w"]), ff2T=w16(inputs["ff2_w"]), clfT=clfT)

    in_maps = []
    for k in range(cfg.NC):
        xslice = np.zeros((128, cfg.NPC), dtype=np.float16)
        xslice[:, :cfg.PCR] = x[k * cfg.PCR:(k + 1) * cfg.PCR].T.astype(np.float16)
        im = dict(xg=xg, xT=xslice, idx=plan["idx"][k], mmoh=plan["mmoh"][k],
                  invdeg=plan["inv"][k], nreal=plan["nreal"][k], par=par,
                  **weights)
        in_maps.append(im)
    return cfg, nc, in_maps


_LAST_EXEC_NS = None
_LAST_RESULTS = None


def _ensure_axon_hooks():
    """The image's antenv lacks axon_hooks; shim it so NTFF tracing works."""
    import sys
    import types
    try:
        import antenv.axon_hooks  # noqa: F401
        return
    except ImportError:
        pass
    import antenv
    mod = types.ModuleType("antenv.axon_hooks")
    mod._hook = None

    def set_axon_ntff_profile_hook(h):
        mod._hook = h

    def get_axon_ntff_profile_hook():
        return mod._hook

    mod.set_axon_ntff_profile_hook = set_axon_ntff_profile_hook
    mod.get_axon_ntff_profile_hook = get_axon_ntff_profile_hook
    sys.modules["antenv.axon_hooks"] = mod
    antenv.axon_hooks = mod
    try:
        from trn_agent_boot.trn_boot import _ntff_profile_via_ctypes
        so = "/opt/axon/libaxon_pjrt.so"
        if os.path.exists(so):
            mod._hook = _ntff_profile_via_ctypes(so)
    except Exception as e:  # pragma: no cover
        print("ntff hook shim failed:", e)


def kernel(**inputs) -> np.ndarray:
    global _LAST_EXEC_NS, _LAST_RESULTS
    cfg, nc, in_maps = _prepare(inputs)
    trace = bool(int(os.environ.get("BASS_GNN_TRACE", "0")))
    if trace:
        _ensure_axon_hooks()
    res = bass_utils.run_bass_kernel_spmd(
        nc, in_maps, core_ids=list(range(cfg.NC)), trace=trace)
    _LAST_EXEC_NS = res.exec_time_ns
    _LAST_RESULTS = res
    out = np.empty((cfg.N, 64), dtype=np.float32)
    for k in range(cfg.NC):
        out[k * cfg.PCR:(k + 1) * cfg.PCR] = res.results[k]["out"][:cfg.PCR]
    return out

